# revision 6
# baseline (speedup 1.0000x reference)
"""GATv2 backbone (4 layers) on 8 Trainium2 NeuronCores — v2.

Design:
  * Nodes partitioned into 8 contiguous edge-balanced ranges; within a core,
    dst nodes grouped into windows of 127. Each window occupies a 128-row
    block of the gathered-feature table; row 127 is a zero dummy so the
    edge-weight term can ride the expand matmul's last contraction slot.
  * Edges owned by the dst core, grouped per dst window, split by
    src-table-row parity, sorted by src, padded to 128-edge tiles (pad slots
    gather table row 0 and carry zero one-hot columns).
  * xl = h @ Wl + bl AllGathered into a full DRAM table per layer; per-edge
    xl[src] rows fetched with dma_gather (SWDGE InstDMAGatherAnt) in 8-tile
    (1024-row) chunks on 2 SWDGE queues. int16 gather indices address the
    52k-row table through pair-stride (2-row) even/odd views.
  * One-hot expand (S', [node k -> edge e]) and scatter (O, [edge e ->
    node n]) matrices precomputed on host, streamed per window as bf16.
    S' row 127 = edge_weight, xr row 127 = We (via the brr bias trick), so
    ep = S'^T @ xr folds the edge-attr projection into one matmul.
  * e_pre = gathered + ep via one DVE add from PSUM (no identity matmul);
    leaky_relu as x + relu(-0.8x); edge intermediates in bf16.
  * Softmax denominators and weighted scatters are matmuls against O.
  * Graph-LayerNorm stats via per-window node->graph one-hot matmuls
    accumulated in PSUM, AllReduced across cores (2x50 floats).
"""

import contextlib

import ml_dtypes
import numpy as np

from concourse import bass, bacc, mybir, tile
from concourse.bass_utils import run_bass_kernel_spmd

P = 128
W = 127            # real nodes per window
NCORES = 8
GMAX = 50          # graphs
HEADS = 4
DHID = 128
CH = DHID // HEADS          # 32
DF = 512                    # final per-head concat width (4*128)
NEG = 0.2
EPS = 1e-5
GCH = 8            # gather chunk: tiles per dma_gather (1024 descriptors)
FCH = 4            # final-layer gather chunk (4 tiles x 512B rows)

F32 = mybir.dt.float32
BF = mybir.dt.bfloat16
I16 = mybir.dt.int16
AX = mybir.AxisListType
OP = mybir.AluOpType
AF = mybir.ActivationFunctionType


# ----------------------------------------------------------------------------
# Host preprocessing: graph partitioning + static schedule
# ----------------------------------------------------------------------------

def build_meta(edge_index, batch):
    N = batch.shape[0]
    E = edge_index.shape[1]
    src = np.asarray(edge_index[0], dtype=np.int64)
    dst = np.asarray(edge_index[1], dtype=np.int64)
    batch = np.asarray(batch, dtype=np.int64)

    deg = np.bincount(dst, minlength=N)
    cum = np.concatenate([[0], np.cumsum(deg)])      # edges with dst < n
    bounds = [0]
    for c in range(1, NCORES):
        n = int(np.searchsorted(cum, c * E / NCORES))
        bounds.append(min(max(n, bounds[-1] + 1), N - (NCORES - c)))
    bounds.append(N)
    lo = np.array(bounds[:-1])
    hi = np.array(bounds[1:])

    NW = int(max((hi - lo + W - 1) // W))
    NPAD = NW * P
    NTOT = NCORES * NPAD
    assert NTOT // 2 <= 32768, "int16 pair-index overflow"

    # node -> table row (row 127 of each 128-block is a dummy)
    trow = np.zeros(N, np.int64)
    core_of = np.zeros(N, np.int64)
    for c in range(NCORES):
        r = np.arange(hi[c] - lo[c])
        trow[lo[c]:hi[c]] = c * NPAD + (r // W) * P + (r % W)
        core_of[lo[c]:hi[c]] = c

    ecore = core_of[dst]
    ewin = (dst - lo[ecore]) // W
    edstl = (dst - lo[ecore]) % W
    epar = (trow[src] % 2).astype(np.int64)

    # per (core, window, parity) counts -> shared tile layout (max over cores)
    cnt = np.zeros((NCORES, NW, 2), np.int64)
    np.add.at(cnt, (ecore, ewin, epar), 1)
    Te = (cnt[:, :, 0].max(axis=0) + P - 1) // P
    To = (cnt[:, :, 1].max(axis=0) + P - 1) // P
    Tw = np.maximum(1, Te + To)
    toff = np.concatenate([[0], np.cumsum(Tw)])
    TT = int(toff[-1])
    Tmax = int(Tw.max())

    # gather chunk schedule (shared across cores): (parity, abs_tile, ntiles)
    chunks = []
    for w in range(NW):
        cw = []
        for a in range(0, int(Te[w]), GCH):
            cw.append((0, int(toff[w] + a), int(min(GCH, Te[w] - a))))
        for a in range(0, int(To[w]), GCH):
            cw.append((1, int(toff[w] + Te[w] + a), int(min(GCH, To[w] - a))))
        chunks.append(cw)
    fchunks = []
    for w in range(NW):
        cw = []
        for a in range(0, int(Te[w]), FCH):
            cw.append((0, int(toff[w] + a), int(min(FCH, Te[w] - a))))
        for a in range(0, int(To[w]), FCH):
            cw.append((1, int(toff[w] + Te[w] + a), int(min(FCH, To[w] - a))))
        fchunks.append(cw)

    # per-core edge slot assignment
    okey = np.lexsort((trow[src], epar, ewin, ecore))
    sc, sw, sp_, = ecore[okey], ewin[okey], epar[okey]
    gkey = (sc * NW + sw) * 2 + sp_
    first = np.zeros(len(gkey), bool)
    first[0] = True
    first[1:] = gkey[1:] != gkey[:-1]
    gstart = np.zeros(len(gkey), np.int64)
    gstart[first] = np.arange(len(gkey))[first]
    gstart = np.maximum.accumulate(gstart)
    j = np.arange(len(gkey)) - gstart                  # rank within group
    base = toff[sw] + np.where(sp_ == 1, Te[sw], 0)    # group tile base
    tilea = base + j // P                              # absolute tile
    posa = j % P

    percore = []
    for c in range(NCORES):
        m = sc == c
        ids = okey[m]
        percore.append(dict(
            eid=ids, tile=tilea[m], pos=posa[m], j=j[m], base=base[m],
            dstl=edstl[ids], ival=(trow[src[ids]] >> 1).astype(np.int16)))

    # graph one-hots per (core, window): [NW, 128, GMAX]
    gmat = np.zeros((NCORES, NW, P, GMAX), np.float32)
    for c in range(NCORES):
        nreal = int(hi[c] - lo[c])
        r = np.arange(nreal)
        gmat[c, r // W, r % W, batch[lo[c]:hi[c]]] = 1.0
    gmatT = np.ascontiguousarray(np.swapaxes(gmat, 2, 3))

    cntg = np.bincount(batch, minlength=GMAX).astype(np.float32)
    invd = (1.0 / (np.maximum(cntg, 1.0) * DHID)).reshape(1, GMAX)

    return dict(N=N, E=E, NW=NW, NPAD=NPAD, NTOT=NTOT, TT=TT, Tmax=Tmax,
                Tw=Tw.astype(int), toff=toff.astype(int),
                Te=Te.astype(int), To=To.astype(int),
                chunks=chunks, fchunks=fchunks,
                lo=lo, hi=hi, percore=percore,
                gmat=gmat, gmatT=gmatT, invd=invd)


# ----------------------------------------------------------------------------
# Bass program
# ----------------------------------------------------------------------------

def build_program(meta):
    NW, NPAD, NTOT, TT = meta["NW"], meta["NPAD"], meta["NTOT"], meta["TT"]
    Tw, toff, Tmax = meta["Tw"], meta["toff"], meta["Tmax"]
    chunks, fchunks = meta["chunks"], meta["fchunks"]

    nc = bacc.Bacc("TRN2", target_bir_lowering=False, debug=False,
                   enable_asserts=False, num_devices=NCORES,
                   num_swdge_queues=2)

    # --- external I/O (per core) ---
    h0s = nc.dram_tensor("h0s", [NPAD, P], BF, kind="ExternalInput")
    rs = nc.dram_tensor("rs", [NPAD, P], F32, kind="ExternalInput")
    idx_d = nc.dram_tensor("idx", [P, 8 * TT], I16, kind="ExternalInput")
    sp_d = nc.dram_tensor("sp", [P, TT * P], BF, kind="ExternalInput")
    ob_d = nc.dram_tensor("ob", [P, TT * P], BF, kind="ExternalInput")
    gmat_d = nc.dram_tensor("gmat", [NW, P, GMAX], F32, kind="ExternalInput")
    gmatT_d = nc.dram_tensor("gmatT", [NW, GMAX, P], F32, kind="ExternalInput")
    invd_d = nc.dram_tensor("invd", [1, GMAX], F32, kind="ExternalInput")
    idf_d = nc.dram_tensor("idf", [P, P], F32, kind="ExternalInput")
    idb_d = nc.dram_tensor("idb", [P, P], BF, kind="ExternalInput")

    wl_d = nc.dram_tensor("wl", [3, P, P], BF, kind="ExternalInput")
    wr_d = nc.dram_tensor("wr", [3, P, P], BF, kind="ExternalInput")
    blr_d = nc.dram_tensor("blr", [3, P, P], F32, kind="ExternalInput")
    brr_d = nc.dram_tensor("brr", [3, P, P], F32, kind="ExternalInput")
    attr_d = nc.dram_tensor("attr", [3, P, P], F32, kind="ExternalInput")
    lnw_d = nc.dram_tensor("lnw", [3, P, P], F32, kind="ExternalInput")
    lnb_d = nc.dram_tensor("lnb", [3, P, P], F32, kind="ExternalInput")
    bia_d = nc.dram_tensor("bia", [3, P, P], F32, kind="ExternalInput")

    wlf_d = nc.dram_tensor("wlf", [P, DF], BF, kind="ExternalInput")
    wrf_d = nc.dram_tensor("wrf", [P, DF], BF, kind="ExternalInput")
    blfr_d = nc.dram_tensor("blfr", [P, DF], F32, kind="ExternalInput")
    brfr_d = nc.dram_tensor("brfr", [P, DF], F32, kind="ExternalInput")
    attfr_d = nc.dram_tensor("attfr", [P, DF], F32, kind="ExternalInput")
    biafr_d = nc.dram_tensor("biafr", [P, P], F32, kind="ExternalInput")

    out_d = nc.dram_tensor("out", [NPAD, P], F32, kind="ExternalOutput")

    qctr = [0]

    with tile.TileContext(nc) as tc, contextlib.ExitStack() as ctx:
        dram = ctx.enter_context(tc.tile_pool(name="dram", bufs=1, space="DRAM"))
        cst = ctx.enter_context(tc.tile_pool(name="cst", bufs=1))
        per = ctx.enter_context(tc.tile_pool(name="per", bufs=1))
        wsp = ctx.enter_context(tc.tile_pool(name="wsp", bufs=2))
        spo = ctx.enter_context(tc.tile_pool(name="spo", bufs=2))
        gpo = ctx.enter_context(tc.tile_pool(name="gpo", bufs=2))
        gpf = ctx.enter_context(tc.tile_pool(name="gpf", bufs=3))

        xl_b = dram.tile([NPAD, P], BF)
        xl_full = dram.tile([NTOT, P], BF)
        xlf_b = dram.tile([NPAD, DF], BF)
        xlf_full = dram.tile([NTOT, DF], BF)
        st_b = dram.tile([2, GMAX], F32)
        st_o = dram.tile([2, GMAX], F32)
        groups = [list(range(NCORES))]

        # --- constants ---
        ident = cst.tile([P, P], F32)
        nc.sync.dma_start(out=ident[:], in_=idf_d[:, :])
        identb = cst.tile([P, P], BF)
        nc.sync.dma_start(out=identb[:], in_=idb_d[:, :])
        epsc = cst.tile([P, 1], F32)
        nc.vector.memset(epsc[:], EPS)
        invd = cst.tile([1, GMAX], F32)
        nc.sync.dma_start(out=invd[:], in_=invd_d[:, :])
        idx_s = cst.tile([P, 8 * TT], I16)
        nc.sync.dma_start(out=idx_s[:], in_=idx_d[:, :])

        # persistent per-layer node-state (window-major)
        h_a = per.tile([P, NW, P], BF, tag="h_a")
        hT = per.tile([P, NW, P], BF, tag="hT")
        htmp = per.tile([P, NW, P], F32, tag="htmp")

        for w in range(NW):
            nc.sync.dma_start(out=h_a[:, w, :], in_=h0s[w * P:(w + 1) * P, :])

        # zero-init rotating gather buffers (stale reads on empty windows)
        for _ in range(2):
            g0 = gpo.tile([P, GCH, P], BF, tag="gq", name="gqz")
            nc.vector.memset(g0[:], 0.0)
        for _ in range(3):
            g1 = gpf.tile([P, FCH, DF], BF, tag="fgq", name="fgqz")
            nc.vector.memset(g1[:], 0.0)

        # pair-stride gather views (int16 indices address 2-row pairs)
        pv = xl_full.opt().rearrange("(a b) c -> a (b c)", b=2)
        v_ev, v_od = pv[:, 0:P], pv[:, P:2 * P]
        pvf = xlf_full.opt().rearrange("(a b) c -> a (b c)", b=2)
        vf_ev, vf_od = pvf[:, 0:DF], pvf[:, DF:2 * DF]

        def gather_window(w, gq_of_tile, views, width, fch, chl):
            """Issue gather chunks for window w; fills gq_of_tile map."""
            for (par, tlo, nt) in chl[w]:
                gq = (gpo.tile([P, GCH, P], BF, tag="gq", name="gq")
                      if width == P else
                      gpf.tile([P, FCH, DF], BF, tag="fgq", name="fgq"))
                nc.gpsimd.dma_gather(
                    gq[:, :nt, :], views[par],
                    idx_s[:, 8 * tlo:8 * (tlo + nt)],
                    nt * P, nt * P, width,
                    elem_step=2 * width, queue_num=qctr[0] % 2)
                qctr[0] += 1
                for t in range(nt):
                    gq_of_tile[tlo + t] = (gq, t)

        # ------------------------------------------------------------------
        def hidden_layer(li, add_resid):
            wl = cst.tile([P, P], BF, tag="wlc", name="wl_t")
            nc.sync.dma_start(out=wl[:], in_=wl_d[li])
            wr = cst.tile([P, P], BF, tag="wrc", name="wr_t")
            nc.sync.dma_start(out=wr[:], in_=wr_d[li])
            blr = cst.tile([P, P], F32, tag="blrc", name="blr_t")
            nc.sync.dma_start(out=blr[:], in_=blr_d[li])
            brr = cst.tile([P, P], F32, tag="brrc", name="brr_t")
            nc.sync.dma_start(out=brr[:], in_=brr_d[li])
            attr = cst.tile([P, P], F32, tag="attrc", name="attr_t")
            nc.sync.dma_start(out=attr[:], in_=attr_d[li])
            lnw = cst.tile([P, P], F32, tag="lnwc", name="lnw_t")
            nc.sync.dma_start(out=lnw[:], in_=lnw_d[li])
            lnb = cst.tile([P, P], F32, tag="lnbc", name="lnb_t")
            nc.sync.dma_start(out=lnb[:], in_=lnb_d[li])
            bia = cst.tile([P, P], F32, tag="biac", name="bia_t")
            nc.sync.dma_start(out=bia[:], in_=bia_d[li])

            with tc.tile_pool(name=f"ps{li}", bufs=1, space="PSUM") as ps:
                # PSUM: ep(1x2) + nmr(1) + dnm(1) + stats(1) + pt(1) + px(1)
                # P0: transposes + xl shard -> DRAM bounce
                for w in range(NW):
                    tp = ps.tile([P, P], BF, space="PSUM", tag="pt",
                                 name="ptb")
                    nc.tensor.transpose(out=tp[:], in_=h_a[:, w, :],
                                        identity=identb[:])
                    nc.vector.tensor_copy(out=hT[:, w, :], in_=tp[:])
                    xp = ps.tile([P, P], F32, space="PSUM", tag="px",
                                 name="px")
                    nc.tensor.matmul(out=xp[:], lhsT=hT[:, w, :], rhs=wl[:],
                                     start=True, stop=True)
                    xs = wsp.tile([P, P], BF, tag="p0xs", name="xs")
                    nc.vector.tensor_tensor(out=xs[:], in0=xp[:], in1=blr[:],
                                            op=OP.add)
                    nc.sync.dma_start(out=xl_b[w * P:(w + 1) * P, :], in_=xs[:])

                # P1: AllGather xl
                nc.gpsimd.collective_compute(
                    "AllGather", OP.bypass, replica_groups=groups,
                    ins=[xl_b.opt()], outs=[xl_full.opt()])

                # P2: edge pipeline per window
                stp = ps.tile([2, GMAX], F32, space="PSUM", tag="stats",
                              name="stp")
                for w in range(NW):
                    T = int(Tw[w])
                    t0 = int(toff[w])
                    xrp = ps.tile([P, P], F32, space="PSUM", tag="px",
                                  name="xrp")
                    nc.tensor.matmul(out=xrp[:], lhsT=hT[:, w, :], rhs=wr[:],
                                     start=True, stop=True)
                    xr = wsp.tile([P, P], BF, tag="xr", name="xr")
                    nc.vector.tensor_tensor(out=xr[:], in0=xrp[:], in1=brr[:],
                                            op=OP.add)
                    spw = spo.tile([P, Tmax * P], BF, tag="sp", name="spw")
                    nc.sync.dma_start(out=spw[:, :T * P],
                                      in_=sp_d[:, t0 * P:(t0 + T) * P])
                    obw = spo.tile([P, Tmax * P], BF, tag="ob", name="obw")
                    nc.scalar.dma_start(out=obw[:, :T * P],
                                        in_=ob_d[:, t0 * P:(t0 + T) * P])
                    gqm = {}
                    gather_window(w, gqm, (v_ev, v_od), P, GCH, chunks)

                    nmr = ps.tile([P, P], F32, space="PSUM", tag="nmr",
                                  name="nmr")
                    dnm = ps.tile([P, HEADS], F32, space="PSUM", tag="dnm",
                                  name="dnm")

                    nq = (T + 3) // 4
                    for q in range(nq):
                        Q = min(4, T - q * 4)
                        ts = q * 4
                        ep = ps.tile([P, 4 * P], F32, space="PSUM", tag="ep",
                                     bufs=2, name="ep")
                        for t in range(Q):
                            nc.tensor.matmul(
                                out=ep[:, t * P:(t + 1) * P],
                                lhsT=spw[:, (ts + t) * P:(ts + t + 1) * P],
                                rhs=xr[:], start=True, stop=True)
                        # gq slices for this quad (may span 2 gather chunks)
                        gq0, r0 = gqm[t0 + ts]
                        contig = all(
                            gqm[t0 + ts + t][0] is gq0
                            and gqm[t0 + ts + t][1] == r0 + t
                            for t in range(Q))
                        if contig:
                            gqv = gq0[:, r0:r0 + Q, :]
                        else:
                            gqc = wsp.tile([P, 4, P], BF, tag="gqc",
                                           name="gqc")
                            for t in range(Q):
                                gt, rt = gqm[t0 + ts + t]
                                nc.vector.tensor_copy(out=gqc[:, t, :],
                                                      in_=gt[:, rt, :])
                            gqv = gqc[:, :Q, :]
                        tq = wsp.tile([P, 4 * P], BF, tag="tq", name="tq")
                        nc.vector.tensor_tensor(
                            out=tq[:, :Q * P], in0=ep[:, :Q * P],
                            in1=gqv.rearrange("p t c -> p (t c)"), op=OP.add)
                        rl = wsp.tile([P, 4 * P], BF, tag="rl", name="rl")
                        nc.scalar.activation(out=rl[:, :Q * P],
                                             in_=tq[:, :Q * P], func=AF.Relu,
                                             scale=-(1.0 - NEG))
                        ea = wsp.tile([P, 4 * P], BF, tag="ea", name="ea")
                        nc.vector.tensor_tensor(out=ea[:, :Q * P],
                                                in0=tq[:, :Q * P],
                                                in1=rl[:, :Q * P], op=OP.add)
                        lg = wsp.tile([P, 4 * P], BF, tag="lg", name="lg")
                        nc.vector.tensor_tensor(
                            out=lg[:, :Q * P], in0=ea[:, :Q * P],
                            in1=attr[:, None, :].to_broadcast([P, Q, P]),
                            op=OP.mult)
                        lgr = wsp.tile([P, 4 * HEADS], F32, tag="lgr",
                                       name="lgr")
                        nc.vector.tensor_reduce(
                            out=lgr[:, :Q * HEADS],
                            in_=lg[:].rearrange("p (t h c) -> p (t h) c",
                                                h=HEADS, c=CH)[:, :Q * HEADS, :],
                            axis=AX.X, op=OP.add)
                        wq = wsp.tile([P, 4 * HEADS], BF, tag="wq", name="wq")
                        nc.scalar.activation(out=wq[:, :Q * HEADS],
                                             in_=lgr[:, :Q * HEADS], func=AF.Exp)
                        mm = wsp.tile([P, 4, HEADS, CH], BF, tag="mm",
                                      name="mmt")
                        nc.vector.tensor_tensor(
                            out=mm[:, :Q, :, :],
                            in0=gqv.rearrange("p t (h c) -> p t h c",
                                              h=HEADS, c=CH),
                            in1=wq[:].rearrange("p (t h) -> p t h", h=HEADS)
                                [:, :Q, :, None].to_broadcast([P, Q, HEADS, CH]),
                            op=OP.mult)
                        for t in range(Q):
                            first = (q == 0 and t == 0)
                            last = (q == nq - 1 and t == Q - 1)
                            ob_t = obw[:, (ts + t) * P:(ts + t + 1) * P]
                            nc.tensor.matmul(
                                out=nmr[:], lhsT=ob_t, rhs=mm[:, t, :, :],
                                start=first, stop=last)
                            nc.tensor.matmul(
                                out=dnm[:], lhsT=ob_t,
                                rhs=wq[:, t * HEADS:(t + 1) * HEADS],
                                start=first, stop=last)

                    # window flush (node-major)
                    rd = wsp.tile([P, HEADS], F32, tag="rd", name="rd")
                    nc.vector.tensor_scalar(out=rd[:], in0=dnm[:],
                                            scalar1=1e-16, scalar2=None,
                                            op0=OP.add)
                    nc.vector.reciprocal(out=rd[:], in_=rd[:])
                    oT = wsp.tile([P, HEADS, CH], F32, tag="oT", name="oT")
                    nc.vector.tensor_tensor(
                        out=oT[:],
                        in0=nmr[:].rearrange("p (h c) -> p h c", h=HEADS, c=CH),
                        in1=rd[:, :, None].to_broadcast([P, HEADS, CH]),
                        op=OP.mult)
                    nc.vector.tensor_tensor(
                        out=htmp[:, w, :],
                        in0=oT[:].rearrange("p h c -> p (h c)"),
                        in1=bia[:], op=OP.add)
                    # stats: [row-sum | row-sumsq] -> per-graph (PSUM accum)
                    s12 = wsp.tile([P, 2], F32, tag="s12", name="s12")
                    nc.vector.tensor_reduce(out=s12[:, 0:1], in_=htmp[:, w, :],
                                            axis=AX.X, op=OP.add)
                    sqj = wsp.tile([P, P], F32, tag="sqj", name="sqj")
                    nc.scalar.activation(out=sqj[:], in_=htmp[:, w, :],
                                         func=AF.Square, accum_out=s12[:, 1:2])
                    gm = wsp.tile([P, GMAX], F32, tag="gm", name="gm")
                    nc.sync.dma_start(out=gm[:], in_=gmat_d[w])
                    nc.tensor.matmul(out=stp[:, :], lhsT=s12[:],
                                     rhs=gm[:], start=(w == 0),
                                     stop=(w == NW - 1))

                # P3: stats -> mean/rstd -> normalize + elu
                sts = wsp.tile([2, GMAX], F32, tag="sts", name="sts")
                nc.vector.tensor_copy(out=sts[:], in_=stp[:])
                nc.sync.dma_start(out=st_b[:, :], in_=sts[:])
                nc.gpsimd.collective_compute(
                    "AllReduce", OP.add, replica_groups=groups,
                    ins=[st_b.opt()], outs=[st_o.opt()])
                stg1 = wsp.tile([1, GMAX], F32, tag="stg1", name="stg1")
                nc.sync.dma_start(out=stg1[:], in_=st_o[0:1, :])
                stg2 = wsp.tile([1, GMAX], F32, tag="stg2", name="stg2")
                nc.sync.dma_start(out=stg2[:], in_=st_o[1:2, :])
                mean = wsp.tile([1, GMAX], F32, tag="mean", name="mean")
                nc.vector.tensor_tensor(out=mean[:], in0=stg1[:],
                                        in1=invd[:], op=OP.mult)
                ex2 = wsp.tile([1, GMAX], F32, tag="ex2", name="ex2")
                nc.vector.tensor_tensor(out=ex2[:], in0=stg2[:],
                                        in1=invd[:], op=OP.mult)
                msq = wsp.tile([1, GMAX], F32, tag="msq", name="msq")
                nc.scalar.activation(out=msq[:], in_=mean[:], func=AF.Square)
                var = wsp.tile([1, GMAX], F32, tag="var", name="var")
                nc.vector.tensor_tensor(out=var[:], in0=ex2[:], in1=msq[:],
                                        op=OP.subtract)
                sd = wsp.tile([1, GMAX], F32, tag="sd", name="sd")
                nc.scalar.activation(out=sd[:], in_=var[:], func=AF.Sqrt,
                                     bias=epsc[0:1, 0:1])
                rstd = wsp.tile([1, GMAX], F32, tag="rstd", name="rstd")
                nc.vector.reciprocal(out=rstd[:], in_=sd[:])
                nmr2 = wsp.tile([1, GMAX], F32, tag="nmr2", name="nm2")
                nc.vector.tensor_tensor(out=nmr2[:], in0=mean[:], in1=rstd[:],
                                        op=OP.mult)
                nc.vector.tensor_scalar(out=nmr2[:], in0=nmr2[:], scalar1=-1.0,
                                        scalar2=None, op0=OP.mult)
                t1 = ps.tile([P, P], F32, space="PSUM", tag="pt", name="t1")
                nc.tensor.transpose(out=t1[0:GMAX, 0:1], in_=nmr2[:],
                                    identity=ident[0:1, 0:1])
                t2 = ps.tile([P, P], F32, space="PSUM", tag="px", name="t2")
                nc.tensor.transpose(out=t2[0:GMAX, 0:1], in_=rstd[:],
                                    identity=ident[0:1, 0:1])
                nrcol = wsp.tile([GMAX, 2], F32, tag="nrcol", name="nrc")
                nc.vector.tensor_copy(out=nrcol[:, 0:1], in_=t1[0:GMAX, 0:1])
                nc.vector.tensor_copy(out=nrcol[:, 1:2], in_=t2[0:GMAX, 0:1])

                for w in range(NW):
                    gmT = wsp.tile([GMAX, P], F32, tag="gmT", name="gmT")
                    nc.sync.dma_start(out=gmT[:], in_=gmatT_d[w])
                    mw = ps.tile([P, P], F32, space="PSUM", tag="pt",
                                 name="mw")
                    nc.tensor.matmul(out=mw[:, 0:2], lhsT=gmT[:], rhs=nrcol[:],
                                     start=True, stop=True)
                    mws = wsp.tile([P, 2], F32, tag="mws", name="mws")
                    nc.vector.tensor_copy(out=mws[:], in_=mw[:, 0:2])
                    xn = wsp.tile([P, P], F32, tag="xn", name="xn")
                    nc.scalar.activation(out=xn[:], in_=htmp[:, w, :],
                                         func=AF.Identity, scale=mws[:, 1:2],
                                         bias=mws[:, 0:1])
                    nc.vector.tensor_tensor(out=xn[:], in0=xn[:], in1=lnw[:],
                                            op=OP.mult)
                    nc.vector.tensor_tensor(out=xn[:], in0=xn[:], in1=lnb[:],
                                            op=OP.add)
                    # elu = max(x,0) + exp(min(x,0)) - 1
                    mn = wsp.tile([P, P], F32, tag="mn", name="mn")
                    nc.vector.tensor_scalar(out=mn[:], in0=xn[:], scalar1=0.0,
                                            scalar2=None, op0=OP.min)
                    nc.scalar.activation(out=mn[:], in_=mn[:], func=AF.Exp)
                    mx = wsp.tile([P, P], F32, tag="mx", name="mx")
                    nc.vector.tensor_scalar(out=mx[:], in0=xn[:], scalar1=0.0,
                                            scalar2=None, op0=OP.max)
                    nc.vector.tensor_tensor(out=mx[:], in0=mx[:], in1=mn[:],
                                            op=OP.add)
                    if add_resid:
                        nc.vector.tensor_scalar(out=mx[:], in0=mx[:],
                                                scalar1=1.0, scalar2=None,
                                                op0=OP.subtract)
                        rt = wsp.tile([P, P], F32, tag="rt", name="rt")
                        nc.sync.dma_start(out=rt[:],
                                          in_=rs[w * P:(w + 1) * P, :])
                        nc.vector.tensor_tensor(out=h_a[0:W, w, :],
                                                in0=mx[0:W, :],
                                                in1=rt[0:W, :], op=OP.add)
                    else:
                        nc.vector.tensor_scalar(out=h_a[0:W, w, :],
                                                in0=mx[0:W, :],
                                                scalar1=1.0, scalar2=None,
                                                op0=OP.subtract)

        # ------------------------------------------------------------------
        def final_layer():
            wlf = cst.tile([P, DF], BF, tag="wlf", name="wlf_t")
            nc.sync.dma_start(out=wlf[:], in_=wlf_d[:, :])
            wrf = cst.tile([P, DF], BF, tag="wrf", name="wrf_t")
            nc.sync.dma_start(out=wrf[:], in_=wrf_d[:, :])
            blfr = cst.tile([P, DF], F32, tag="blfr", name="blf_t")
            nc.sync.dma_start(out=blfr[:], in_=blfr_d[:, :])
            brfr = cst.tile([P, DF], F32, tag="brfr", name="brf_t")
            nc.sync.dma_start(out=brfr[:], in_=brfr_d[:, :])
            attfr = cst.tile([P, DF], F32, tag="attfr", name="atf_t")
            nc.sync.dma_start(out=attfr[:], in_=attfr_d[:, :])
            biafr = cst.tile([P, P], F32, tag="biafr", name="biaf_t")
            nc.sync.dma_start(out=biafr[:], in_=biafr_d[:, :])

            with tc.tile_pool(name="psf", bufs=1, space="PSUM") as ps:
                # PSUM: fep(1x2) + fnm(1) + fdnm(1) + fpt(1)
                for w in range(NW):
                    tp = ps.tile([P, P], BF, space="PSUM", tag="fpt",
                                 name="ftpb")
                    nc.tensor.transpose(out=tp[:], in_=h_a[:, w, :],
                                        identity=identb[:])
                    nc.vector.tensor_copy(out=hT[:, w, :], in_=tp[:])
                    xp = ps.tile([P, DF], F32, space="PSUM", tag="fep",
                                 bufs=2, name="fxp")
                    nc.tensor.matmul(out=xp[:], lhsT=hT[:, w, :], rhs=wlf[:],
                                     start=True, stop=True)
                    xs = wsp.tile([P, DF], BF, tag="fxs", bufs=1, name="fxs")
                    nc.vector.tensor_tensor(out=xs[:], in0=xp[:], in1=blfr[:],
                                            op=OP.add)
                    nc.sync.dma_start(out=xlf_b[w * P:(w + 1) * P, :],
                                      in_=xs[:])

                nc.gpsimd.collective_compute(
                    "AllGather", OP.bypass, replica_groups=groups,
                    ins=[xlf_b.opt()], outs=[xlf_full.opt()])

                for w in range(NW):
                    T = int(Tw[w])
                    t0 = int(toff[w])
                    xrp = ps.tile([P, DF], F32, space="PSUM", tag="fep",
                                  bufs=2, name="fxrp")
                    nc.tensor.matmul(out=xrp[:], lhsT=hT[:, w, :], rhs=wrf[:],
                                     start=True, stop=True)
                    xr = wsp.tile([P, DF], BF, tag="fxr", bufs=1, name="fxr")
                    nc.vector.tensor_tensor(out=xr[:], in0=xrp[:], in1=brfr[:],
                                            op=OP.add)
                    spw = spo.tile([P, Tmax * P], BF, tag="sp", name="fspw")
                    nc.sync.dma_start(out=spw[:, :T * P],
                                      in_=sp_d[:, t0 * P:(t0 + T) * P])
                    obw = spo.tile([P, Tmax * P], BF, tag="ob", name="fobw")
                    nc.scalar.dma_start(out=obw[:, :T * P],
                                        in_=ob_d[:, t0 * P:(t0 + T) * P])
                    gqm = {}
                    gather_window(w, gqm, (vf_ev, vf_od), DF, FCH, fchunks)

                    fnm = ps.tile([P, DF], F32, space="PSUM", tag="fnm",
                                  name="fnm")
                    dnm = ps.tile([P, HEADS], F32, space="PSUM", tag="fdnm",
                                  name="fdnm")

                    for t in range(T):
                        gqb, rt_ = gqm[t0 + t]
                        gqv = gqb[:, rt_, :]
                        ep = ps.tile([P, DF], F32, space="PSUM", tag="fep",
                                     bufs=2, name="fept")
                        nc.tensor.matmul(out=ep[:],
                                         lhsT=spw[:, t * P:(t + 1) * P],
                                         rhs=xr[:], start=True, stop=True)
                        tq = wsp.tile([P, DF], BF, tag="ftq", name="ftq")
                        nc.vector.tensor_tensor(out=tq[:], in0=ep[:],
                                                in1=gqv, op=OP.add)
                        rl = wsp.tile([P, DF], BF, tag="frl", name="frl")
                        nc.scalar.activation(out=rl[:], in_=tq[:],
                                             func=AF.Relu,
                                             scale=-(1.0 - NEG))
                        ea = wsp.tile([P, DF], BF, tag="fea", name="fea")
                        nc.vector.tensor_tensor(out=ea[:], in0=tq[:],
                                                in1=rl[:], op=OP.add)
                        lg = wsp.tile([P, DF], BF, tag="flg", name="flg")
                        nc.vector.tensor_tensor(out=lg[:], in0=ea[:],
                                                in1=attfr[:], op=OP.mult)
                        lgr = wsp.tile([P, HEADS], F32, tag="flgr",
                                       name="flgr")
                        nc.vector.tensor_reduce(
                            out=lgr[:],
                            in_=lg[:].rearrange("p (h c) -> p h c", h=HEADS,
                                                c=P),
                            axis=AX.X, op=OP.add)
                        wq = wsp.tile([P, HEADS], BF, tag="fwq", name="fwq")
                        nc.scalar.activation(out=wq[:], in_=lgr[:], func=AF.Exp)
                        mm = wsp.tile([P, HEADS, P], BF, tag="fmm", bufs=2,
                                      name="fmm")
                        nc.vector.tensor_tensor(
                            out=mm[:],
                            in0=gqv.rearrange("p (h c) -> p h c", h=HEADS,
                                              c=P),
                            in1=wq[:, :, None].to_broadcast([P, HEADS, P]),
                            op=OP.mult)
                        ob_t = obw[:, t * P:(t + 1) * P]
                        nc.tensor.matmul(
                            out=fnm[:], lhsT=ob_t,
                            rhs=mm[:].rearrange("p h c -> p (h c)"),
                            start=(t == 0), stop=(t == T - 1))
                        nc.tensor.matmul(out=dnm[:], lhsT=ob_t, rhs=wq[:],
                                         start=(t == 0), stop=(t == T - 1))

                    # flush: out = bias + sum_h numer[n,h,:]*(0.25/denom[n,h])
                    rd = wsp.tile([P, HEADS], F32, tag="rd", name="rdf")
                    nc.vector.tensor_scalar(out=rd[:], in0=dnm[:],
                                            scalar1=1e-16, scalar2=None,
                                            op0=OP.add)
                    nc.vector.reciprocal(out=rd[:], in_=rd[:])
                    nc.vector.tensor_scalar(out=rd[:], in0=rd[:],
                                            scalar1=1.0 / HEADS, scalar2=None,
                                            op0=OP.mult)
                    sc = wsp.tile([P, HEADS, P], F32, tag="sc", bufs=1,
                                  name="sc")
                    nc.vector.tensor_tensor(
                        out=sc[:],
                        in0=fnm[:].rearrange("p (h c) -> p h c", h=HEADS, c=P),
                        in1=rd[:, :, None].to_broadcast([P, HEADS, P]),
                        op=OP.mult)
                    acc = wsp.tile([P, P], F32, tag="acc", name="acc")
                    nc.vector.tensor_reduce(
                        out=acc[:], in_=sc[:].rearrange("p h c -> p c h"),
                        axis=AX.X, op=OP.add)
                    nc.vector.tensor_tensor(out=acc[:], in0=acc[:],
                                            in1=biafr[:], op=OP.add)
                    nc.sync.dma_start(out=out_d[w * P:(w + 1) * P, :],
                                      in_=acc[:])

        # ---- the 4 layers ----
        hidden_layer(0, add_resid=False)
        hidden_layer(1, add_resid=True)
        hidden_layer(2, add_resid=False)
        final_layer()

    nc.compile()
    return nc


# ----------------------------------------------------------------------------
# Host-side driver
# ----------------------------------------------------------------------------

def _rep(v, rows=P):
    v = np.asarray(v, np.float32).reshape(-1)
    return np.broadcast_to(v, (rows, v.shape[0])).copy()


def make_in_maps(meta, inputs):
    NPAD, TT, NW = meta["NPAD"], meta["TT"], meta["NW"]
    lo, hi = meta["lo"], meta["hi"]
    x = np.asarray(inputs["x"], np.float32)
    resid = np.asarray(inputs["residual"], np.float32)
    ew = np.asarray(inputs["edge_weight"], np.float32)

    att = np.asarray(inputs["att"], np.float32)        # (3, H, C)
    attf = np.asarray(inputs["att_f"], np.float32)     # (H, DOUT)
    We = np.asarray(inputs["We"], np.float32)          # (3, 1, DHID)
    Wef = np.asarray(inputs["We_f"], np.float32)       # (1, H*DOUT)

    brr = np.stack([_rep(inputs["br"][i]) for i in range(3)])
    for i in range(3):
        brr[i, P - 1, :] = We[i, 0, :]
    brfr = _rep(inputs["br_f"])
    brfr[P - 1, :] = Wef[0, :]

    common = dict(
        invd=meta["invd"].astype(np.float32),
        idf=np.eye(P, dtype=np.float32),
        idb=np.eye(P, dtype=np.float32).astype(ml_dtypes.bfloat16),
        wl=np.asarray(inputs["Wl"], np.float32).astype(ml_dtypes.bfloat16),
        wr=np.asarray(inputs["Wr"], np.float32).astype(ml_dtypes.bfloat16),
        blr=np.stack([_rep(inputs["bl"][i]) for i in range(3)]),
        brr=brr,
        attr=np.stack([_rep(att[i]) for i in range(3)]),
        lnw=np.stack([_rep(inputs["ln_w"][i]) for i in range(3)]),
        lnb=np.stack([_rep(inputs["ln_b"][i]) for i in range(3)]),
        bia=np.stack([_rep(inputs["bias"][i]) for i in range(3)]),
        wlf=np.asarray(inputs["Wl_f"], np.float32).astype(ml_dtypes.bfloat16),
        wrf=np.asarray(inputs["Wr_f"], np.float32).astype(ml_dtypes.bfloat16),
        blfr=_rep(inputs["bl_f"]),
        brfr=brfr,
        attfr=_rep(attf),
        biafr=_rep(inputs["bias_f"]),
    )

    in_maps = []
    for c in range(NCORES):
        n = int(hi[c] - lo[c])
        r = np.arange(n)
        prow = (r // W) * P + (r % W)
        h0s = np.zeros((NPAD, P), ml_dtypes.bfloat16)
        h0s[prow] = x[lo[c]:hi[c]].astype(ml_dtypes.bfloat16)
        rss = np.zeros((NPAD, P), np.float32)
        rss[prow] = resid[lo[c]:hi[c]]

        pc = meta["percore"][c]
        col = pc["tile"] * P
        sp = np.zeros((P, TT * P), ml_dtypes.bfloat16)
        sp[pc["dstl"], col + pc["pos"]] = 1.0
        sp[P - 1, col + pc["pos"]] = ew[pc["eid"]].astype(ml_dtypes.bfloat16)
        ob = np.zeros((P, TT * P), ml_dtypes.bfloat16)
        ob[pc["pos"], col + pc["dstl"]] = 1.0
        idx16 = np.zeros((16, 8 * TT), np.int16)
        icol = 8 * pc["base"] + pc["j"] // 16
        irow = pc["j"] % 16
        idx16[irow, icol] = pc["ival"]
        idx = np.tile(idx16, (8, 1))

        in_maps.append(dict(
            h0s=h0s, rs=rss, idx=idx, sp=sp, ob=ob,
            gmat=meta["gmat"][c], gmatT=meta["gmatT"][c],
            **common))
    return in_maps


def assemble(meta, results):
    N = meta["N"]
    lo, hi = meta["lo"], meta["hi"]
    out = np.zeros((N, P), np.float32)
    for c in range(NCORES):
        n = int(hi[c] - lo[c])
        r = np.arange(n)
        out[lo[c]:hi[c]] = results[c]["out"][(r // W) * P + (r % W)]
    return out


_CACHE = {}


def kernel(**inputs):
    ei = np.asarray(inputs["edge_index"])
    bt = np.asarray(inputs["batch"])
    key = (ei.shape, bt.shape, hash(ei.tobytes()), hash(bt.tobytes()))
    if key not in _CACHE:
        meta = build_meta(ei, bt)
        nc = build_program(meta)
        _CACHE[key] = (meta, nc)
    meta, nc = _CACHE[key]
    in_maps = make_in_maps(meta, inputs)
    res = run_bass_kernel_spmd(nc, in_maps, list(range(NCORES)))
    return assemble(meta, res.results)


# revision 11
# speedup vs baseline: 1.5171x; 1.5171x over previous
"""GATv2 backbone (4 layers) on 8 Trainium2 NeuronCores — v2.

Design:
  * Nodes partitioned into 8 contiguous edge-balanced ranges; within a core,
    dst nodes grouped into windows of 127. Each window occupies a 128-row
    block of the gathered-feature table; row 127 is a zero dummy so the
    edge-weight term can ride the expand matmul's last contraction slot.
  * Edges owned by the dst core, grouped per dst window, split by
    src-table-row parity, sorted by src, padded to 128-edge tiles (pad slots
    gather table row 0 and carry zero one-hot columns).
  * xl = h @ Wl + bl AllGathered into a full DRAM table per layer; per-edge
    xl[src] rows fetched with dma_gather (SWDGE InstDMAGatherAnt) in 8-tile
    (1024-row) chunks on 2 SWDGE queues. int16 gather indices address the
    52k-row table through pair-stride (2-row) even/odd views.
  * One-hot expand (S', [node k -> edge e]) and scatter (O, [edge e ->
    node n]) matrices precomputed on host, streamed per window as bf16.
    S' row 127 = edge_weight, xr row 127 = We (via the brr bias trick), so
    ep = S'^T @ xr folds the edge-attr projection into one matmul.
  * e_pre = gathered + ep via one DVE add from PSUM (no identity matmul);
    leaky_relu as x + relu(-0.8x); edge intermediates in bf16.
  * Softmax denominators and weighted scatters are matmuls against O.
  * Graph-LayerNorm stats via per-window node->graph one-hot matmuls
    accumulated in PSUM, AllReduced across cores (2x50 floats).
"""

import contextlib

import ml_dtypes
import numpy as np

from concourse import bass, bacc, mybir, tile
from concourse.bass_utils import run_bass_kernel_spmd

P = 128
W = 127            # real nodes per window
NCORES = 8
GMAX = 50          # graphs
HEADS = 4
DHID = 128
CH = DHID // HEADS          # 32
DF = 512                    # final per-head concat width (4*128)
NEG = 0.2
EPS = 1e-5
GCH = 8            # gather chunk: tiles per dma_gather (1024 descriptors)
FCH = 4            # final-layer gather chunk (4 tiles x 512B rows)

F32 = mybir.dt.float32
BF = mybir.dt.bfloat16
I16 = mybir.dt.int16
AX = mybir.AxisListType
OP = mybir.AluOpType
AF = mybir.ActivationFunctionType


# ----------------------------------------------------------------------------
# Host preprocessing: graph partitioning + static schedule
# ----------------------------------------------------------------------------

def build_meta(edge_index, batch):
    N = batch.shape[0]
    E = edge_index.shape[1]
    src = np.asarray(edge_index[0], dtype=np.int64)
    dst = np.asarray(edge_index[1], dtype=np.int64)
    batch = np.asarray(batch, dtype=np.int64)

    deg = np.bincount(dst, minlength=N)
    cum = np.concatenate([[0], np.cumsum(deg)])      # edges with dst < n
    bounds = [0]
    for c in range(1, NCORES):
        n = int(np.searchsorted(cum, c * E / NCORES))
        bounds.append(min(max(n, bounds[-1] + 1), N - (NCORES - c)))
    bounds.append(N)
    lo = np.array(bounds[:-1])
    hi = np.array(bounds[1:])

    NW = int(max((hi - lo + W - 1) // W))
    NPAD = NW * P
    NTOT = NCORES * NPAD
    assert NTOT // 2 <= 32768, "int16 pair-index overflow"

    # node -> table row (row 127 of each 128-block is a dummy)
    trow = np.zeros(N, np.int64)
    core_of = np.zeros(N, np.int64)
    for c in range(NCORES):
        r = np.arange(hi[c] - lo[c])
        trow[lo[c]:hi[c]] = c * NPAD + (r // W) * P + (r % W)
        core_of[lo[c]:hi[c]] = c

    ecore = core_of[dst]
    ewin = (dst - lo[ecore]) // W
    edstl = (dst - lo[ecore]) % W
    epar = (trow[src] % 2).astype(np.int64)

    # per (core, window, parity) counts -> shared tile layout (max over cores)
    cnt = np.zeros((NCORES, NW, 2), np.int64)
    np.add.at(cnt, (ecore, ewin, epar), 1)
    Te = (cnt[:, :, 0].max(axis=0) + P - 1) // P
    To = (cnt[:, :, 1].max(axis=0) + P - 1) // P
    Tw = np.maximum(1, Te + To)
    toff = np.concatenate([[0], np.cumsum(Tw)])
    TT = int(toff[-1])
    Tmax = int(Tw.max())

    # gather chunk schedule (shared across cores): (parity, abs_tile, ntiles)
    chunks = []
    for w in range(NW):
        cw = []
        for a in range(0, int(Te[w]), GCH):
            cw.append((0, int(toff[w] + a), int(min(GCH, Te[w] - a))))
        for a in range(0, int(To[w]), GCH):
            cw.append((1, int(toff[w] + Te[w] + a), int(min(GCH, To[w] - a))))
        chunks.append(cw)
    fchunks = []
    for w in range(NW):
        cw = []
        for a in range(0, int(Te[w]), FCH):
            cw.append((0, int(toff[w] + a), int(min(FCH, Te[w] - a))))
        for a in range(0, int(To[w]), FCH):
            cw.append((1, int(toff[w] + Te[w] + a), int(min(FCH, To[w] - a))))
        fchunks.append(cw)

    # per-core edge slot assignment
    okey = np.lexsort((trow[src], epar, ewin, ecore))
    sc, sw, sp_, = ecore[okey], ewin[okey], epar[okey]
    gkey = (sc * NW + sw) * 2 + sp_
    first = np.zeros(len(gkey), bool)
    first[0] = True
    first[1:] = gkey[1:] != gkey[:-1]
    gstart = np.zeros(len(gkey), np.int64)
    gstart[first] = np.arange(len(gkey))[first]
    gstart = np.maximum.accumulate(gstart)
    j = np.arange(len(gkey)) - gstart                  # rank within group
    base = toff[sw] + np.where(sp_ == 1, Te[sw], 0)    # group tile base
    tilea = base + j // P                              # absolute tile
    posa = j % P

    percore = []
    for c in range(NCORES):
        m = sc == c
        ids = okey[m]
        percore.append(dict(
            eid=ids, tile=tilea[m], pos=posa[m], j=j[m], base=base[m],
            dstl=edstl[ids], ival=(trow[src[ids]] >> 1).astype(np.int16)))

    # graph one-hots per (core, window): [NW, 128, GMAX]
    gmat = np.zeros((NCORES, NW, P, GMAX), np.float32)
    for c in range(NCORES):
        nreal = int(hi[c] - lo[c])
        r = np.arange(nreal)
        gmat[c, r // W, r % W, batch[lo[c]:hi[c]]] = 1.0
    gmatT = np.ascontiguousarray(np.swapaxes(gmat, 2, 3))

    cntg = np.bincount(batch, minlength=GMAX).astype(np.float32)
    invd = (1.0 / (np.maximum(cntg, 1.0) * DHID)).reshape(1, GMAX)

    return dict(N=N, E=E, NW=NW, NPAD=NPAD, NTOT=NTOT, TT=TT, Tmax=Tmax,
                Tw=Tw.astype(int), toff=toff.astype(int),
                Te=Te.astype(int), To=To.astype(int),
                chunks=chunks, fchunks=fchunks,
                lo=lo, hi=hi, percore=percore,
                gmat=gmat, gmatT=gmatT, invd=invd)


# ----------------------------------------------------------------------------
# Bass program
# ----------------------------------------------------------------------------

def build_program(meta):
    NW, NPAD, NTOT, TT = meta["NW"], meta["NPAD"], meta["NTOT"], meta["TT"]
    Tw, toff, Tmax = meta["Tw"], meta["toff"], meta["Tmax"]
    chunks, fchunks = meta["chunks"], meta["fchunks"]

    nc = bacc.Bacc("TRN2", target_bir_lowering=False, debug=False,
                   enable_asserts=False, num_devices=NCORES,
                   num_swdge_queues=2)

    # --- external I/O (per core) ---
    h0s = nc.dram_tensor("h0s", [NPAD, P], BF, kind="ExternalInput")
    rs = nc.dram_tensor("rs", [NPAD, P], F32, kind="ExternalInput")
    idx_d = nc.dram_tensor("idx", [P, 8 * TT], I16, kind="ExternalInput")
    sp_d = nc.dram_tensor("sp", [P, TT * P], BF, kind="ExternalInput")
    ob_d = nc.dram_tensor("ob", [P, TT * P], BF, kind="ExternalInput")
    gmat_d = nc.dram_tensor("gmat", [NW, P, GMAX], F32, kind="ExternalInput")
    gmatT_d = nc.dram_tensor("gmatT", [NW, GMAX, P], F32, kind="ExternalInput")
    invd_d = nc.dram_tensor("invd", [1, GMAX], F32, kind="ExternalInput")
    idf_d = nc.dram_tensor("idf", [P, P], F32, kind="ExternalInput")
    idb_d = nc.dram_tensor("idb", [P, P], BF, kind="ExternalInput")

    wl_d = nc.dram_tensor("wl", [3, P, P], BF, kind="ExternalInput")
    wr_d = nc.dram_tensor("wr", [3, P, P], BF, kind="ExternalInput")
    blr_d = nc.dram_tensor("blr", [3, P, P], F32, kind="ExternalInput")
    brr_d = nc.dram_tensor("brr", [3, P, P], F32, kind="ExternalInput")
    attr_d = nc.dram_tensor("attr", [3, P, P], BF, kind="ExternalInput")
    lnw_d = nc.dram_tensor("lnw", [3, P, P], F32, kind="ExternalInput")
    lnb_d = nc.dram_tensor("lnb", [3, P, P], F32, kind="ExternalInput")
    bia_d = nc.dram_tensor("bia", [3, P, P], F32, kind="ExternalInput")

    wlf_d = nc.dram_tensor("wlf", [P, DF], BF, kind="ExternalInput")
    wrf_d = nc.dram_tensor("wrf", [P, DF], BF, kind="ExternalInput")
    blfr_d = nc.dram_tensor("blfr", [P, DF], F32, kind="ExternalInput")
    brfr_d = nc.dram_tensor("brfr", [P, DF], F32, kind="ExternalInput")
    attfr_d = nc.dram_tensor("attfr", [P, DF], BF, kind="ExternalInput")
    biafr_d = nc.dram_tensor("biafr", [P, P], F32, kind="ExternalInput")

    out_d = nc.dram_tensor("out", [NPAD, P], F32, kind="ExternalOutput")

    qctr = [0]

    with tile.TileContext(nc) as tc, contextlib.ExitStack() as ctx:
        dram = ctx.enter_context(tc.tile_pool(name="dram", bufs=1, space="DRAM"))
        cst = ctx.enter_context(tc.tile_pool(name="cst", bufs=1))
        per = ctx.enter_context(tc.tile_pool(name="per", bufs=1))
        wsp = ctx.enter_context(tc.tile_pool(name="wsp", bufs=2))
        spo = ctx.enter_context(tc.tile_pool(name="spo", bufs=2))
        gpo = ctx.enter_context(tc.tile_pool(name="gpo", bufs=6))
        gpf = ctx.enter_context(tc.tile_pool(name="gpf", bufs=5))

        xl_b = dram.tile([NPAD, P], BF)
        xl_full = dram.tile([NTOT, P], BF)
        xlf_b = dram.tile([NPAD, DF], BF)
        xlf_full = dram.tile([NTOT, DF], BF)
        st_b = dram.tile([2, GMAX], F32)
        st_o = dram.tile([2, GMAX], F32)
        groups = [list(range(NCORES))]

        # --- constants ---
        ident = cst.tile([P, P], F32)
        nc.sync.dma_start(out=ident[:], in_=idf_d[:, :])
        identb = cst.tile([P, P], BF)
        nc.sync.dma_start(out=identb[:], in_=idb_d[:, :])
        epsc = cst.tile([P, 1], F32)
        nc.vector.memset(epsc[:], EPS)
        invd = cst.tile([1, GMAX], F32)
        nc.sync.dma_start(out=invd[:], in_=invd_d[:, :])
        idx_s = cst.tile([P, 8 * TT], I16)
        nc.sync.dma_start(out=idx_s[:], in_=idx_d[:, :])

        # persistent per-layer node-state (window-major)
        h_a = per.tile([P, NW, P], BF, tag="h_a")
        hT = per.tile([P, NW, P], BF, tag="hT")
        htmp = per.tile([P, NW, P], F32, tag="htmp")

        for w in range(NW):
            nc.sync.dma_start(out=h_a[:, w, :], in_=h0s[w * P:(w + 1) * P, :])

        # zero-init rotating gather buffers (stale reads on empty windows)
        for _ in range(6):
            g0 = gpo.tile([P, GCH, P], BF, tag="gq", name="gqz")
            nc.vector.memset(g0[:], 0.0)
        for _ in range(5):
            g1 = gpf.tile([P, FCH, DF], BF, tag="fgq", name="fgqz")
            nc.vector.memset(g1[:], 0.0)

        # pair-stride gather views (int16 indices address 2-row pairs)
        pv = xl_full.opt().rearrange("(a b) c -> a (b c)", b=2)
        v_ev, v_od = pv[:, 0:P], pv[:, P:2 * P]
        pvf = xlf_full.opt().rearrange("(a b) c -> a (b c)", b=2)
        vf_ev, vf_od = pvf[:, 0:DF], pvf[:, DF:2 * DF]

        def gather_window(w, gq_of_tile, views, width, fch, chl):
            """Issue gather chunks for window w; fills gq_of_tile map."""
            out = []
            for (par, tlo, nt) in chl[w]:
                gq = (gpo.tile([P, GCH, P], BF, tag="gq", name="gq")
                      if width == P else
                      gpf.tile([P, FCH, DF], BF, tag="fgq", name="fgq"))
                nc.gpsimd.dma_gather(
                    gq[:, :nt, :], views[par],
                    idx_s[:, 8 * tlo:8 * (tlo + nt)],
                    nt * P, nt * P, width,
                    elem_step=2 * width, queue_num=qctr[0] % 2)
                qctr[0] += 1
                for t in range(nt):
                    gq_of_tile[tlo + t] = (gq, t)
                out.append(((par, tlo, nt), gq))
            return out

        # ------------------------------------------------------------------
        def hidden_layer(li, add_resid):
            wl = cst.tile([P, P], BF, tag="wlc", name="wl_t")
            nc.sync.dma_start(out=wl[:], in_=wl_d[li])
            wr = cst.tile([P, P], BF, tag="wrc", name="wr_t")
            nc.sync.dma_start(out=wr[:], in_=wr_d[li])
            blr = cst.tile([P, P], F32, tag="blrc", name="blr_t")
            nc.sync.dma_start(out=blr[:], in_=blr_d[li])
            brr = cst.tile([P, P], F32, tag="brrc", name="brr_t")
            nc.sync.dma_start(out=brr[:], in_=brr_d[li])
            attr = cst.tile([P, P], BF, tag="attrc", name="attr_t")
            nc.sync.dma_start(out=attr[:], in_=attr_d[li])
            lnw = cst.tile([P, P], F32, tag="lnwc", name="lnw_t")
            nc.sync.dma_start(out=lnw[:], in_=lnw_d[li])
            lnb = cst.tile([P, P], F32, tag="lnbc", name="lnb_t")
            nc.sync.dma_start(out=lnb[:], in_=lnb_d[li])
            bia = cst.tile([P, P], F32, tag="biac", name="bia_t")
            nc.sync.dma_start(out=bia[:], in_=bia_d[li])

            with tc.tile_pool(name=f"ps{li}", bufs=1, space="PSUM") as ps:
                # PSUM: ep(1x2) + nmr(1) + dnm(1) + stats(1) + pt(1) + px(1)
                # P0: transposes + xl shard -> DRAM bounce
                for w in range(NW):
                    tp = ps.tile([P, P], BF, space="PSUM", tag="pt",
                                 name="ptb")
                    nc.tensor.transpose(out=tp[:], in_=h_a[:, w, :],
                                        identity=identb[:])
                    nc.vector.tensor_copy(out=hT[:, w, :], in_=tp[:])
                    xp = ps.tile([P, P], F32, space="PSUM", tag="px",
                                 name="px")
                    nc.tensor.matmul(out=xp[:], lhsT=hT[:, w, :], rhs=wl[:],
                                     start=True, stop=True)
                    xs = wsp.tile([P, P], BF, tag="p0xs", name="xs")
                    nc.vector.tensor_tensor(out=xs[:], in0=xp[:], in1=blr[:],
                                            op=OP.add)
                    nc.sync.dma_start(out=xl_b[w * P:(w + 1) * P, :], in_=xs[:])

                # P1: AllGather xl
                nc.gpsimd.collective_compute(
                    "AllGather", OP.bypass, replica_groups=groups,
                    ins=[xl_b.opt()], outs=[xl_full.opt()])

                # P2: edge pipeline per window
                stp = ps.tile([2, GMAX], F32, space="PSUM", tag="stats",
                              name="stp")
                for w in range(NW):
                    T = int(Tw[w])
                    t0 = int(toff[w])
                    xrp = ps.tile([P, P], F32, space="PSUM", tag="px",
                                  name="xrp")
                    nc.tensor.matmul(out=xrp[:], lhsT=hT[:, w, :], rhs=wr[:],
                                     start=True, stop=True)
                    xr = wsp.tile([P, P], BF, tag="xr", name="xr")
                    nc.vector.tensor_tensor(out=xr[:], in0=xrp[:], in1=brr[:],
                                            op=OP.add)
                    spw = spo.tile([P, Tmax * P], BF, tag="sp", name="spw")
                    nc.sync.dma_start(out=spw[:, :T * P],
                                      in_=sp_d[:, t0 * P:(t0 + T) * P])
                    obw = spo.tile([P, Tmax * P], BF, tag="ob", name="obw")
                    nc.scalar.dma_start(out=obw[:, :T * P],
                                        in_=ob_d[:, t0 * P:(t0 + T) * P])
                    gqm = {}
                    gtiles = gather_window(w, gqm, (v_ev, v_od), P, GCH,
                                           chunks)
                    quads = []          # (gq buffer, rel_start, Q, abs_tile)
                    for (par, tlo, nt), gq in gtiles:
                        for a in range(0, nt, 4):
                            quads.append((gq, a, min(4, nt - a), tlo + a))

                    nmr = ps.tile([P, P], F32, space="PSUM", tag="nmr",
                                  name="nmr")
                    dnm = ps.tile([P, HEADS], F32, space="PSUM", tag="dnm",
                                  name="dnm")

                    for qi, (gqb, a, Q, tabs) in enumerate(quads):
                        ts = tabs - t0
                        ep = ps.tile([P, 4 * P], F32, space="PSUM", tag="ep",
                                     bufs=2, name="ep")
                        for t in range(Q):
                            nc.tensor.matmul(
                                out=ep[:, t * P:(t + 1) * P],
                                lhsT=spw[:, (ts + t) * P:(ts + t + 1) * P],
                                rhs=xr[:], start=True, stop=True)
                        gqv = gqb[:, a:a + Q, :]
                        tq = wsp.tile([P, 4 * P], BF, tag="tq", name="tq")
                        nc.vector.tensor_tensor(
                            out=tq[:, :Q * P], in0=ep[:, :Q * P],
                            in1=gqv.rearrange("p t c -> p (t c)"), op=OP.add)
                        rl = wsp.tile([P, 4 * P], BF, tag="rl", name="rl")
                        nc.scalar.activation(out=rl[:, :Q * P],
                                             in_=tq[:, :Q * P], func=AF.Relu,
                                             scale=-(1.0 - NEG))
                        ea = wsp.tile([P, 4 * P], BF, tag="ea", name="ea")
                        nc.vector.tensor_tensor(out=ea[:, :Q * P],
                                                in0=tq[:, :Q * P],
                                                in1=rl[:, :Q * P], op=OP.add)
                        lg = wsp.tile([P, 4 * P], BF, tag="lg", name="lg")
                        nc.vector.tensor_tensor(
                            out=lg[:, :Q * P], in0=ea[:, :Q * P],
                            in1=attr[:, None, :].to_broadcast([P, Q, P]),
                            op=OP.mult)
                        lgr = wsp.tile([P, 4 * HEADS], BF, tag="lgr",
                                       name="lgr")
                        nc.vector.tensor_reduce(
                            out=lgr[:, :Q * HEADS],
                            in_=lg[:].rearrange("p (t h c) -> p (t h) c",
                                                h=HEADS, c=CH)[:, :Q * HEADS, :],
                            axis=AX.X, op=OP.add)
                        wq = wsp.tile([P, 4 * HEADS], BF, tag="wq", name="wq")
                        nc.scalar.activation(out=wq[:, :Q * HEADS],
                                             in_=lgr[:, :Q * HEADS], func=AF.Exp)
                        mm = wsp.tile([P, 4, HEADS, CH], BF, tag="mm",
                                      name="mmt")
                        nc.vector.tensor_tensor(
                            out=mm[:, :Q, :, :],
                            in0=gqv.rearrange("p t (h c) -> p t h c",
                                              h=HEADS, c=CH),
                            in1=wq[:].rearrange("p (t h) -> p t h", h=HEADS)
                                [:, :Q, :, None].to_broadcast([P, Q, HEADS, CH]),
                            op=OP.mult)
                        for t in range(Q):
                            first = (qi == 0 and t == 0)
                            last = (qi == len(quads) - 1 and t == Q - 1)
                            ob_t = obw[:, (ts + t) * P:(ts + t + 1) * P]
                            nc.tensor.matmul(
                                out=nmr[:], lhsT=ob_t, rhs=mm[:, t, :, :],
                                start=first, stop=last)
                            nc.tensor.matmul(
                                out=dnm[:], lhsT=ob_t,
                                rhs=wq[:, t * HEADS:(t + 1) * HEADS],
                                start=first, stop=last)

                    # window flush (node-major)
                    if not quads:
                        nc.vector.tensor_copy(out=htmp[:, w, :], in_=bia[:])
                    if quads:
                        rd = wsp.tile([P, HEADS], F32, tag="rd", name="rd")
                        nc.vector.tensor_scalar(out=rd[:], in0=dnm[:],
                                                scalar1=1e-16, scalar2=None,
                                                op0=OP.add)
                        nc.vector.reciprocal(out=rd[:], in_=rd[:])
                        oT = wsp.tile([P, HEADS, CH], F32, tag="oT", name="oT")
                        nc.vector.tensor_tensor(
                            out=oT[:],
                            in0=nmr[:].rearrange("p (h c) -> p h c",
                                                 h=HEADS, c=CH),
                            in1=rd[:, :, None].to_broadcast([P, HEADS, CH]),
                            op=OP.mult)
                        nc.vector.tensor_tensor(
                            out=htmp[:, w, :],
                            in0=oT[:].rearrange("p h c -> p (h c)"),
                            in1=bia[:], op=OP.add)
                    # stats: [row-sum | row-sumsq] -> per-graph (PSUM accum)
                    s12 = wsp.tile([P, 2], F32, tag="s12", name="s12")
                    nc.vector.tensor_reduce(out=s12[:, 0:1], in_=htmp[:, w, :],
                                            axis=AX.X, op=OP.add)
                    sqj = wsp.tile([P, P], F32, tag="sqj", name="sqj")
                    nc.scalar.activation(out=sqj[:], in_=htmp[:, w, :],
                                         func=AF.Square, accum_out=s12[:, 1:2])
                    gm = wsp.tile([P, GMAX], F32, tag="gm", name="gm")
                    nc.sync.dma_start(out=gm[:], in_=gmat_d[w])
                    nc.tensor.matmul(out=stp[:, :], lhsT=s12[:],
                                     rhs=gm[:], start=(w == 0),
                                     stop=(w == NW - 1))

                # P3: stats -> mean/rstd -> normalize + elu
                sts = wsp.tile([2, GMAX], F32, tag="sts", name="sts")
                nc.vector.tensor_copy(out=sts[:], in_=stp[:])
                nc.sync.dma_start(out=st_b[:, :], in_=sts[:])
                nc.gpsimd.collective_compute(
                    "AllReduce", OP.add, replica_groups=groups,
                    ins=[st_b.opt()], outs=[st_o.opt()])
                stg1 = wsp.tile([1, GMAX], F32, tag="stg1", name="stg1")
                nc.sync.dma_start(out=stg1[:], in_=st_o[0:1, :])
                stg2 = wsp.tile([1, GMAX], F32, tag="stg2", name="stg2")
                nc.sync.dma_start(out=stg2[:], in_=st_o[1:2, :])
                mean = wsp.tile([1, GMAX], F32, tag="mean", name="mean")
                nc.vector.tensor_tensor(out=mean[:], in0=stg1[:],
                                        in1=invd[:], op=OP.mult)
                ex2 = wsp.tile([1, GMAX], F32, tag="ex2", name="ex2")
                nc.vector.tensor_tensor(out=ex2[:], in0=stg2[:],
                                        in1=invd[:], op=OP.mult)
                msq = wsp.tile([1, GMAX], F32, tag="msq", name="msq")
                nc.scalar.activation(out=msq[:], in_=mean[:], func=AF.Square)
                var = wsp.tile([1, GMAX], F32, tag="var", name="var")
                nc.vector.tensor_tensor(out=var[:], in0=ex2[:], in1=msq[:],
                                        op=OP.subtract)
                sd = wsp.tile([1, GMAX], F32, tag="sd", name="sd")
                nc.scalar.activation(out=sd[:], in_=var[:], func=AF.Sqrt,
                                     bias=epsc[0:1, 0:1])
                rstd = wsp.tile([1, GMAX], F32, tag="rstd", name="rstd")
                nc.vector.reciprocal(out=rstd[:], in_=sd[:])
                nmr2 = wsp.tile([1, GMAX], F32, tag="nmr2", name="nm2")
                nc.vector.tensor_tensor(out=nmr2[:], in0=mean[:], in1=rstd[:],
                                        op=OP.mult)
                nc.vector.tensor_scalar(out=nmr2[:], in0=nmr2[:], scalar1=-1.0,
                                        scalar2=None, op0=OP.mult)
                t1 = ps.tile([P, P], F32, space="PSUM", tag="pt", name="t1")
                nc.tensor.transpose(out=t1[0:GMAX, 0:1], in_=nmr2[:],
                                    identity=ident[0:1, 0:1])
                t2 = ps.tile([P, P], F32, space="PSUM", tag="px", name="t2")
                nc.tensor.transpose(out=t2[0:GMAX, 0:1], in_=rstd[:],
                                    identity=ident[0:1, 0:1])
                nrcol = wsp.tile([GMAX, 2], F32, tag="nrcol", name="nrc")
                nc.vector.tensor_copy(out=nrcol[:, 0:1], in_=t1[0:GMAX, 0:1])
                nc.vector.tensor_copy(out=nrcol[:, 1:2], in_=t2[0:GMAX, 0:1])

                for w in range(NW):
                    gmT = wsp.tile([GMAX, P], F32, tag="gmT", name="gmT")
                    nc.sync.dma_start(out=gmT[:], in_=gmatT_d[w])
                    mw = ps.tile([P, P], F32, space="PSUM", tag="pt",
                                 name="mw")
                    nc.tensor.matmul(out=mw[:, 0:2], lhsT=gmT[:], rhs=nrcol[:],
                                     start=True, stop=True)
                    mws = wsp.tile([P, 2], F32, tag="mws", name="mws")
                    nc.vector.tensor_copy(out=mws[:], in_=mw[:, 0:2])
                    xn = wsp.tile([P, P], F32, tag="xn", name="xn")
                    nc.scalar.activation(out=xn[:], in_=htmp[:, w, :],
                                         func=AF.Identity, scale=mws[:, 1:2],
                                         bias=mws[:, 0:1])
                    nc.vector.tensor_tensor(out=xn[:], in0=xn[:], in1=lnw[:],
                                            op=OP.mult)
                    nc.vector.tensor_tensor(out=xn[:], in0=xn[:], in1=lnb[:],
                                            op=OP.add)
                    # elu = max(x,0) + exp(min(x,0)) - 1
                    mn = wsp.tile([P, P], F32, tag="mn", name="mn")
                    nc.vector.tensor_scalar(out=mn[:], in0=xn[:], scalar1=0.0,
                                            scalar2=None, op0=OP.min)
                    nc.scalar.activation(out=mn[:], in_=mn[:], func=AF.Exp)
                    mx = wsp.tile([P, P], F32, tag="mx", name="mx")
                    nc.vector.tensor_scalar(out=mx[:], in0=xn[:], scalar1=0.0,
                                            scalar2=None, op0=OP.max)
                    nc.vector.tensor_tensor(out=mx[:], in0=mx[:], in1=mn[:],
                                            op=OP.add)
                    if add_resid:
                        nc.vector.tensor_scalar(out=mx[:], in0=mx[:],
                                                scalar1=1.0, scalar2=None,
                                                op0=OP.subtract)
                        rt = wsp.tile([P, P], F32, tag="rt", name="rt")
                        nc.sync.dma_start(out=rt[:],
                                          in_=rs[w * P:(w + 1) * P, :])
                        nc.vector.tensor_tensor(out=h_a[0:W, w, :],
                                                in0=mx[0:W, :],
                                                in1=rt[0:W, :], op=OP.add)
                    else:
                        nc.vector.tensor_scalar(out=h_a[0:W, w, :],
                                                in0=mx[0:W, :],
                                                scalar1=1.0, scalar2=None,
                                                op0=OP.subtract)

        # ------------------------------------------------------------------
        def final_layer():
            wlf = cst.tile([P, DF], BF, tag="wlf", name="wlf_t")
            nc.sync.dma_start(out=wlf[:], in_=wlf_d[:, :])
            wrf = cst.tile([P, DF], BF, tag="wrf", name="wrf_t")
            nc.sync.dma_start(out=wrf[:], in_=wrf_d[:, :])
            blfr = cst.tile([P, DF], F32, tag="blfr", name="blf_t")
            nc.sync.dma_start(out=blfr[:], in_=blfr_d[:, :])
            brfr = cst.tile([P, DF], F32, tag="brfr", name="brf_t")
            nc.sync.dma_start(out=brfr[:], in_=brfr_d[:, :])
            attfr = cst.tile([P, DF], BF, tag="attfr", name="atf_t")
            nc.sync.dma_start(out=attfr[:], in_=attfr_d[:, :])
            biafr = cst.tile([P, P], F32, tag="biafr", name="biaf_t")
            nc.sync.dma_start(out=biafr[:], in_=biafr_d[:, :])

            with tc.tile_pool(name="psf", bufs=1, space="PSUM") as ps:
                # PSUM: fep(1x2) + fnm(1) + fdnm(1) + fpt(1)
                for w in range(NW):
                    tp = ps.tile([P, P], BF, space="PSUM", tag="fpt",
                                 name="ftpb")
                    nc.tensor.transpose(out=tp[:], in_=h_a[:, w, :],
                                        identity=identb[:])
                    nc.vector.tensor_copy(out=hT[:, w, :], in_=tp[:])
                    xp = ps.tile([P, DF], F32, space="PSUM", tag="fep",
                                 bufs=2, name="fxp")
                    nc.tensor.matmul(out=xp[:], lhsT=hT[:, w, :], rhs=wlf[:],
                                     start=True, stop=True)
                    xs = wsp.tile([P, DF], BF, tag="fxs", bufs=1, name="fxs")
                    nc.vector.tensor_tensor(out=xs[:], in0=xp[:], in1=blfr[:],
                                            op=OP.add)
                    nc.sync.dma_start(out=xlf_b[w * P:(w + 1) * P, :],
                                      in_=xs[:])

                nc.gpsimd.collective_compute(
                    "AllGather", OP.bypass, replica_groups=groups,
                    ins=[xlf_b.opt()], outs=[xlf_full.opt()])

                for w in range(NW):
                    T = int(Tw[w])
                    t0 = int(toff[w])
                    xrp = ps.tile([P, DF], F32, space="PSUM", tag="fep",
                                  bufs=2, name="fxrp")
                    nc.tensor.matmul(out=xrp[:], lhsT=hT[:, w, :], rhs=wrf[:],
                                     start=True, stop=True)
                    xr = wsp.tile([P, DF], BF, tag="fxr", bufs=1, name="fxr")
                    nc.vector.tensor_tensor(out=xr[:], in0=xrp[:], in1=brfr[:],
                                            op=OP.add)
                    spw = spo.tile([P, Tmax * P], BF, tag="sp", name="fspw")
                    nc.sync.dma_start(out=spw[:, :T * P],
                                      in_=sp_d[:, t0 * P:(t0 + T) * P])
                    obw = spo.tile([P, Tmax * P], BF, tag="ob", name="fobw")
                    nc.scalar.dma_start(out=obw[:, :T * P],
                                        in_=ob_d[:, t0 * P:(t0 + T) * P])
                    gqm = {}
                    gather_window(w, gqm, (vf_ev, vf_od), DF, FCH, fchunks)

                    fnm = ps.tile([P, DF], F32, space="PSUM", tag="fnm",
                                  name="fnm")
                    dnm = ps.tile([P, HEADS], F32, space="PSUM", tag="fdnm",
                                  name="fdnm")

                    for t in range(T):
                        gqb, rt_ = gqm[t0 + t]
                        gqv = gqb[:, rt_, :]
                        ep = ps.tile([P, DF], F32, space="PSUM", tag="fept",
                                     bufs=2, name="fept")
                        nc.tensor.matmul(out=ep[:],
                                         lhsT=spw[:, t * P:(t + 1) * P],
                                         rhs=xr[:], start=True, stop=True)
                        cp = wsp.tile([P, DF], BF, tag="fcp", name="fcp")
                        nc.scalar.activation(out=cp[:], in_=ep[:],
                                             func=AF.Identity)
                        tq = wsp.tile([P, DF], BF, tag="ftq", name="ftq")
                        nc.vector.tensor_tensor(out=tq[:], in0=cp[:],
                                                in1=gqv, op=OP.add)
                        rl = wsp.tile([P, DF], BF, tag="frl", name="frl")
                        nc.scalar.activation(out=rl[:], in_=tq[:],
                                             func=AF.Relu,
                                             scale=-(1.0 - NEG))
                        ea = wsp.tile([P, DF], BF, tag="fea", name="fea")
                        nc.vector.tensor_tensor(out=ea[:], in0=tq[:],
                                                in1=rl[:], op=OP.add)
                        lg = wsp.tile([P, DF], BF, tag="flg", name="flg")
                        nc.vector.tensor_tensor(out=lg[:], in0=ea[:],
                                                in1=attfr[:], op=OP.mult)
                        lgr = wsp.tile([P, HEADS], BF, tag="flgr",
                                       name="flgr")
                        nc.vector.tensor_reduce(
                            out=lgr[:],
                            in_=lg[:].rearrange("p (h c) -> p h c", h=HEADS,
                                                c=P),
                            axis=AX.X, op=OP.add)
                        wq = wsp.tile([P, HEADS], BF, tag="fwq", name="fwq")
                        nc.scalar.activation(out=wq[:], in_=lgr[:], func=AF.Exp)
                        mm = wsp.tile([P, HEADS, P], BF, tag="fmm", bufs=2,
                                      name="fmm")
                        nc.vector.tensor_tensor(
                            out=mm[:],
                            in0=gqv.rearrange("p (h c) -> p h c", h=HEADS,
                                              c=P),
                            in1=wq[:, :, None].to_broadcast([P, HEADS, P]),
                            op=OP.mult)
                        ob_t = obw[:, t * P:(t + 1) * P]
                        nc.tensor.matmul(
                            out=fnm[:], lhsT=ob_t,
                            rhs=mm[:].rearrange("p h c -> p (h c)"),
                            start=(t == 0), stop=(t == T - 1))
                        nc.tensor.matmul(out=dnm[:], lhsT=ob_t, rhs=wq[:],
                                         start=(t == 0), stop=(t == T - 1))

                    # flush: out = bias + sum_h numer[n,h,:]*(0.25/denom[n,h])
                    rd = wsp.tile([P, HEADS], F32, tag="rd", name="rdf")
                    nc.vector.tensor_scalar(out=rd[:], in0=dnm[:],
                                            scalar1=1e-16, scalar2=None,
                                            op0=OP.add)
                    nc.vector.reciprocal(out=rd[:], in_=rd[:])
                    nc.vector.tensor_scalar(out=rd[:], in0=rd[:],
                                            scalar1=1.0 / HEADS, scalar2=None,
                                            op0=OP.mult)
                    fns = wsp.tile([P, DF], BF, tag="fns", name="fns")
                    nc.scalar.activation(out=fns[:], in_=fnm[:],
                                         func=AF.Identity)
                    sc = wsp.tile([P, HEADS, P], F32, tag="sc", bufs=1,
                                  name="sc")
                    nc.vector.tensor_tensor(
                        out=sc[:],
                        in0=fns[:].rearrange("p (h c) -> p h c", h=HEADS, c=P),
                        in1=rd[:, :, None].to_broadcast([P, HEADS, P]),
                        op=OP.mult)
                    acc = wsp.tile([P, P], F32, tag="acc", name="acc")
                    nc.vector.tensor_reduce(
                        out=acc[:], in_=sc[:].rearrange("p h c -> p c h"),
                        axis=AX.X, op=OP.add)
                    nc.vector.tensor_tensor(out=acc[:], in0=acc[:],
                                            in1=biafr[:], op=OP.add)
                    nc.sync.dma_start(out=out_d[w * P:(w + 1) * P, :],
                                      in_=acc[:])

        # ---- the 4 layers ----
        with nc.allow_low_precision(reason="bf16 edge intermediates; "
                                    "softmax tolerates it (rel-err gate)"):
            hidden_layer(0, add_resid=False)
            hidden_layer(1, add_resid=True)
            hidden_layer(2, add_resid=False)
            final_layer()

    nc.compile()
    return nc


# ----------------------------------------------------------------------------
# Host-side driver
# ----------------------------------------------------------------------------

def _rep(v, rows=P):
    v = np.asarray(v, np.float32).reshape(-1)
    return np.broadcast_to(v, (rows, v.shape[0])).copy()


def make_in_maps(meta, inputs):
    NPAD, TT, NW = meta["NPAD"], meta["TT"], meta["NW"]
    lo, hi = meta["lo"], meta["hi"]
    x = np.asarray(inputs["x"], np.float32)
    resid = np.asarray(inputs["residual"], np.float32)
    ew = np.asarray(inputs["edge_weight"], np.float32)

    att = np.asarray(inputs["att"], np.float32)        # (3, H, C)
    attf = np.asarray(inputs["att_f"], np.float32)     # (H, DOUT)
    We = np.asarray(inputs["We"], np.float32)          # (3, 1, DHID)
    Wef = np.asarray(inputs["We_f"], np.float32)       # (1, H*DOUT)

    brr = np.stack([_rep(inputs["br"][i]) for i in range(3)])
    for i in range(3):
        brr[i, P - 1, :] = We[i, 0, :]
    brfr = _rep(inputs["br_f"])
    brfr[P - 1, :] = Wef[0, :]

    common = dict(
        invd=meta["invd"].astype(np.float32),
        idf=np.eye(P, dtype=np.float32),
        idb=np.eye(P, dtype=np.float32).astype(ml_dtypes.bfloat16),
        wl=np.asarray(inputs["Wl"], np.float32).astype(ml_dtypes.bfloat16),
        wr=np.asarray(inputs["Wr"], np.float32).astype(ml_dtypes.bfloat16),
        blr=np.stack([_rep(inputs["bl"][i]) for i in range(3)]),
        brr=brr,
        attr=np.stack([_rep(att[i]) for i in range(3)]).astype(ml_dtypes.bfloat16),
        lnw=np.stack([_rep(inputs["ln_w"][i]) for i in range(3)]),
        lnb=np.stack([_rep(inputs["ln_b"][i]) for i in range(3)]),
        bia=np.stack([_rep(inputs["bias"][i]) for i in range(3)]),
        wlf=np.asarray(inputs["Wl_f"], np.float32).astype(ml_dtypes.bfloat16),
        wrf=np.asarray(inputs["Wr_f"], np.float32).astype(ml_dtypes.bfloat16),
        blfr=_rep(inputs["bl_f"]),
        brfr=brfr,
        attfr=_rep(attf).astype(ml_dtypes.bfloat16),
        biafr=_rep(inputs["bias_f"]),
    )

    in_maps = []
    for c in range(NCORES):
        n = int(hi[c] - lo[c])
        r = np.arange(n)
        prow = (r // W) * P + (r % W)
        h0s = np.zeros((NPAD, P), ml_dtypes.bfloat16)
        h0s[prow] = x[lo[c]:hi[c]].astype(ml_dtypes.bfloat16)
        rss = np.zeros((NPAD, P), np.float32)
        rss[prow] = resid[lo[c]:hi[c]]

        pc = meta["percore"][c]
        col = pc["tile"] * P
        sp = np.zeros((P, TT * P), ml_dtypes.bfloat16)
        sp[pc["dstl"], col + pc["pos"]] = 1.0
        sp[P - 1, col + pc["pos"]] = ew[pc["eid"]].astype(ml_dtypes.bfloat16)
        ob = np.zeros((P, TT * P), ml_dtypes.bfloat16)
        ob[pc["pos"], col + pc["dstl"]] = 1.0
        idx16 = np.zeros((16, 8 * TT), np.int16)
        icol = 8 * pc["base"] + pc["j"] // 16
        irow = pc["j"] % 16
        idx16[irow, icol] = pc["ival"]
        idx = np.tile(idx16, (8, 1))

        in_maps.append(dict(
            h0s=h0s, rs=rss, idx=idx, sp=sp, ob=ob,
            gmat=meta["gmat"][c], gmatT=meta["gmatT"][c],
            **common))
    return in_maps


def assemble(meta, results):
    N = meta["N"]
    lo, hi = meta["lo"], meta["hi"]
    out = np.zeros((N, P), np.float32)
    for c in range(NCORES):
        n = int(hi[c] - lo[c])
        r = np.arange(n)
        out[lo[c]:hi[c]] = results[c]["out"][(r // W) * P + (r % W)]
    return out


_CACHE = {}


def kernel(**inputs):
    ei = np.asarray(inputs["edge_index"])
    bt = np.asarray(inputs["batch"])
    key = (ei.shape, bt.shape, hash(ei.tobytes()), hash(bt.tobytes()))
    if key not in _CACHE:
        meta = build_meta(ei, bt)
        nc = build_program(meta)
        _CACHE[key] = (meta, nc)
    meta, nc = _CACHE[key]
    in_maps = make_in_maps(meta, inputs)
    res = run_bass_kernel_spmd(nc, in_maps, list(range(NCORES)))
    return assemble(meta, res.results)


# revision 20
# speedup vs baseline: 1.5316x; 1.0096x over previous
"""GATv2 backbone (4 layers) on 8 Trainium2 NeuronCores — v2.

Design:
  * Nodes partitioned into 8 contiguous edge-balanced ranges; within a core,
    dst nodes grouped into windows of 127. Each window occupies a 128-row
    block of the gathered-feature table; row 127 is a zero dummy so the
    edge-weight term can ride the expand matmul's last contraction slot.
  * Edges owned by the dst core, grouped per dst window, split by
    src-table-row parity, sorted by src, padded to 128-edge tiles (pad slots
    gather table row 0 and carry zero one-hot columns).
  * xl = h @ Wl + bl AllGathered into a full DRAM table per layer; per-edge
    xl[src] rows fetched with dma_gather (SWDGE InstDMAGatherAnt) in 8-tile
    (1024-row) chunks on 2 SWDGE queues. int16 gather indices address the
    52k-row table through pair-stride (2-row) even/odd views.
  * One-hot expand (S', [node k -> edge e]) and scatter (O, [edge e ->
    node n]) matrices precomputed on host, streamed per window as bf16.
    S' row 127 = edge_weight, xr row 127 = We (via the brr bias trick), so
    ep = S'^T @ xr folds the edge-attr projection into one matmul.
  * e_pre = gathered + ep via one DVE add from PSUM (no identity matmul);
    leaky_relu as x + relu(-0.8x); edge intermediates in bf16.
  * Softmax denominators and weighted scatters are matmuls against O.
  * Graph-LayerNorm stats via per-window node->graph one-hot matmuls
    accumulated in PSUM, AllReduced across cores (2x50 floats).
"""

import contextlib

import ml_dtypes
import numpy as np

from concourse import bass, bacc, mybir, tile
from concourse.bass_utils import run_bass_kernel_spmd

P = 128
W = 127            # real nodes per window
NCORES = 8
GMAX = 50          # graphs
HEADS = 4
DHID = 128
CH = DHID // HEADS          # 32
DF = 512                    # final per-head concat width (4*128)
NEG = 0.2
EPS = 1e-5
GCH = 8            # gather chunk: tiles per dma_gather (1024 descriptors)
FCH = 4            # final-layer gather chunk (4 tiles x 512B rows)

F32 = mybir.dt.float32
BF = mybir.dt.bfloat16
I16 = mybir.dt.int16
AX = mybir.AxisListType
OP = mybir.AluOpType
AF = mybir.ActivationFunctionType


# ----------------------------------------------------------------------------
# Host preprocessing: graph partitioning + static schedule
# ----------------------------------------------------------------------------

def build_meta(edge_index, batch):
    N = batch.shape[0]
    E = edge_index.shape[1]
    src = np.asarray(edge_index[0], dtype=np.int64)
    dst = np.asarray(edge_index[1], dtype=np.int64)
    batch = np.asarray(batch, dtype=np.int64)

    deg = np.bincount(dst, minlength=N)
    cum = np.concatenate([[0], np.cumsum(deg)])      # edges with dst < n
    bounds = [0]
    for c in range(1, NCORES):
        n = int(np.searchsorted(cum, c * E / NCORES))
        bounds.append(min(max(n, bounds[-1] + 1), N - (NCORES - c)))
    bounds.append(N)
    lo = np.array(bounds[:-1])
    hi = np.array(bounds[1:])

    NW = int(max((hi - lo + W - 1) // W))
    NPAD = NW * P
    NTOT = NCORES * NPAD
    assert NTOT // 2 <= 32768, "int16 pair-index overflow"

    # AllGather chunking (table is chunk-major so chunked AG lands
    # contiguously): chunk k covers windows [a_k, b_k)
    NAG = 4
    cb = (NW + NAG - 1) // NAG
    agb = [(k * cb, min((k + 1) * cb, NW)) for k in range(NAG)]
    agb = [(a, b) for (a, b) in agb if a < b]
    chunk_of = np.zeros(NW, np.int64)
    chunk_base = np.zeros(NW, np.int64)     # per-window row base in the table
    roff = 0
    for k, (a, b) in enumerate(agb):
        chunk_of[a:b] = k
        for w in range(a, b):
            chunk_base[w] = roff            # filled below per window
        roff += NCORES * (b - a) * P

    # node -> table row (row 127 of each 128-block is a dummy)
    # row(c, w, s) = chunkbase + c*(b-a)*128 + (w-a)*128 + s
    trow = np.zeros(N, np.int64)
    core_of = np.zeros(N, np.int64)
    for c in range(NCORES):
        r = np.arange(hi[c] - lo[c])
        w_ = r // W
        k_ = chunk_of[w_]
        a_ = np.array([agb[k][0] for k in k_])
        b_ = np.array([agb[k][1] for k in k_])
        trow[lo[c]:hi[c]] = (chunk_base[w_] + c * (b_ - a_) * P
                             + (w_ - a_) * P + (r % W))
        core_of[lo[c]:hi[c]] = c

    ecore = core_of[dst]
    ewin = (dst - lo[ecore]) // W
    edstl = (dst - lo[ecore]) % W
    epar = (trow[src] % 2).astype(np.int64)

    # per (core, window, parity) counts -> shared tile layout (max over cores)
    cnt = np.zeros((NCORES, NW, 2), np.int64)
    np.add.at(cnt, (ecore, ewin, epar), 1)
    Te = (cnt[:, :, 0].max(axis=0) + P - 1) // P
    To = (cnt[:, :, 1].max(axis=0) + P - 1) // P
    Tw = np.maximum(1, Te + To)
    toff = np.concatenate([[0], np.cumsum(Tw)])
    TT = int(toff[-1])
    Tmax = int(Tw.max())

    # gather chunk schedule (shared across cores): (parity, abs_tile, ntiles)
    chunks = []
    for w in range(NW):
        cw = []
        for a in range(0, int(Te[w]), GCH):
            cw.append((0, int(toff[w] + a), int(min(GCH, Te[w] - a))))
        for a in range(0, int(To[w]), GCH):
            cw.append((1, int(toff[w] + Te[w] + a), int(min(GCH, To[w] - a))))
        chunks.append(cw)
    fchunks = []
    for w in range(NW):
        cw = []
        for a in range(0, int(Te[w]), FCH):
            cw.append((0, int(toff[w] + a), int(min(FCH, Te[w] - a))))
        for a in range(0, int(To[w]), FCH):
            cw.append((1, int(toff[w] + Te[w] + a), int(min(FCH, To[w] - a))))
        fchunks.append(cw)

    # per-core edge slot assignment
    okey = np.lexsort((trow[src], epar, ewin, ecore))
    sc, sw, sp_, = ecore[okey], ewin[okey], epar[okey]
    gkey = (sc * NW + sw) * 2 + sp_
    first = np.zeros(len(gkey), bool)
    first[0] = True
    first[1:] = gkey[1:] != gkey[:-1]
    gstart = np.zeros(len(gkey), np.int64)
    gstart[first] = np.arange(len(gkey))[first]
    gstart = np.maximum.accumulate(gstart)
    j = np.arange(len(gkey)) - gstart                  # rank within group
    base = toff[sw] + np.where(sp_ == 1, Te[sw], 0)    # group tile base
    tilea = base + j // P                              # absolute tile
    posa = j % P

    percore = []
    for c in range(NCORES):
        m = sc == c
        ids = okey[m]
        percore.append(dict(
            eid=ids, tile=tilea[m], pos=posa[m], j=j[m], base=base[m],
            dstl=edstl[ids], ival=(trow[src[ids]] >> 1).astype(np.int16)))

    # graph one-hots per (core, window): [NW, 128, GMAX]
    gmat = np.zeros((NCORES, NW, P, GMAX), np.float32)
    for c in range(NCORES):
        nreal = int(hi[c] - lo[c])
        r = np.arange(nreal)
        gmat[c, r // W, r % W, batch[lo[c]:hi[c]]] = 1.0
    gmatT = np.ascontiguousarray(np.swapaxes(gmat, 2, 3))

    cntg = np.bincount(batch, minlength=GMAX).astype(np.float32)
    invd = (1.0 / (np.maximum(cntg, 1.0) * DHID)).reshape(1, GMAX)

    return dict(N=N, E=E, NW=NW, NPAD=NPAD, NTOT=NTOT, TT=TT, Tmax=Tmax,
                Tw=Tw.astype(int), toff=toff.astype(int),
                Te=Te.astype(int), To=To.astype(int),
                chunks=chunks, fchunks=fchunks, agb=agb,
                lo=lo, hi=hi, percore=percore,
                gmat=gmat, gmatT=gmatT, invd=invd)


# ----------------------------------------------------------------------------
# Bass program
# ----------------------------------------------------------------------------

def build_program(meta):
    NW, NPAD, NTOT, TT = meta["NW"], meta["NPAD"], meta["NTOT"], meta["TT"]
    Tw, toff, Tmax = meta["Tw"], meta["toff"], meta["Tmax"]
    chunks, fchunks = meta["chunks"], meta["fchunks"]

    nc = bacc.Bacc("TRN2", target_bir_lowering=False, debug=False,
                   enable_asserts=False, num_devices=NCORES,
                   num_swdge_queues=2)

    # --- external I/O (per core) ---
    h0s = nc.dram_tensor("h0s", [NPAD, P], BF, kind="ExternalInput")
    rs = nc.dram_tensor("rs", [NPAD, P], F32, kind="ExternalInput")
    idx_d = nc.dram_tensor("idx", [P, 8 * TT], I16, kind="ExternalInput")
    sp_d = nc.dram_tensor("sp", [P, TT * P], BF, kind="ExternalInput")
    ob_d = nc.dram_tensor("ob", [P, TT * P], BF, kind="ExternalInput")
    gmat_d = nc.dram_tensor("gmat", [NW, P, GMAX], F32, kind="ExternalInput")
    gmatT_d = nc.dram_tensor("gmatT", [NW, GMAX, P], F32, kind="ExternalInput")
    invd_d = nc.dram_tensor("invd", [1, GMAX], F32, kind="ExternalInput")
    idf_d = nc.dram_tensor("idf", [P, P], F32, kind="ExternalInput")
    idb_d = nc.dram_tensor("idb", [P, P], BF, kind="ExternalInput")

    wl_d = nc.dram_tensor("wl", [3, P, P], BF, kind="ExternalInput")
    wr_d = nc.dram_tensor("wr", [3, P, P], BF, kind="ExternalInput")
    blr_d = nc.dram_tensor("blr", [3, P, P], F32, kind="ExternalInput")
    brr_d = nc.dram_tensor("brr", [3, P, P], F32, kind="ExternalInput")
    attr_d = nc.dram_tensor("attr", [3, P, P], BF, kind="ExternalInput")
    lnw_d = nc.dram_tensor("lnw", [3, P, P], F32, kind="ExternalInput")
    lnb_d = nc.dram_tensor("lnb", [3, P, P], F32, kind="ExternalInput")
    bia_d = nc.dram_tensor("bia", [3, P, P], F32, kind="ExternalInput")

    wlf_d = nc.dram_tensor("wlf", [P, DF], BF, kind="ExternalInput")
    wrf_d = nc.dram_tensor("wrf", [P, DF], BF, kind="ExternalInput")
    blfr_d = nc.dram_tensor("blfr", [P, DF], F32, kind="ExternalInput")
    brfr_d = nc.dram_tensor("brfr", [P, DF], F32, kind="ExternalInput")
    attfr_d = nc.dram_tensor("attfr", [P, DF], BF, kind="ExternalInput")
    biafr_d = nc.dram_tensor("biafr", [P, P], F32, kind="ExternalInput")

    out_d = nc.dram_tensor("out", [NPAD, P], F32, kind="ExternalOutput")

    qctr = [0]

    with tile.TileContext(nc) as tc, contextlib.ExitStack() as ctx:
        dram = ctx.enter_context(tc.tile_pool(name="dram", bufs=1, space="DRAM"))
        cst = ctx.enter_context(tc.tile_pool(name="cst", bufs=1))
        per = ctx.enter_context(tc.tile_pool(name="per", bufs=1))
        wsp = ctx.enter_context(tc.tile_pool(name="wsp", bufs=2))
        spo = ctx.enter_context(tc.tile_pool(name="spo", bufs=2))
        gpo = ctx.enter_context(tc.tile_pool(name="gpo", bufs=6))
        gpf = ctx.enter_context(tc.tile_pool(name="gpf", bufs=5))

        xl_b = dram.tile([NPAD, P], BF)
        xl_full = dram.tile([NTOT, P], BF)
        xlf_b = dram.tile([NPAD, DF], BF)
        xlf_full = dram.tile([NTOT, DF], BF)
        st_b = dram.tile([2, GMAX], F32)
        st_o = dram.tile([2, GMAX], F32)
        groups = [list(range(NCORES))]

        # --- constants ---
        ident = cst.tile([P, P], F32)
        nc.sync.dma_start(out=ident[:], in_=idf_d[:, :])
        identb = cst.tile([P, P], BF)
        nc.sync.dma_start(out=identb[:], in_=idb_d[:, :])
        epsc = cst.tile([P, 1], F32)
        nc.vector.memset(epsc[:], EPS)
        invd = cst.tile([1, GMAX], F32)
        nc.sync.dma_start(out=invd[:], in_=invd_d[:, :])
        idx_s = cst.tile([P, 8 * TT], I16)
        nc.sync.dma_start(out=idx_s[:], in_=idx_d[:, :])

        # persistent per-layer node-state (window-major)
        h_a = per.tile([P, NW, P], BF, tag="h_a")
        hT = per.tile([P, NW, P], BF, tag="hT")
        htmp = per.tile([P, NW, P], F32, tag="htmp")

        for w in range(NW):
            nc.sync.dma_start(out=h_a[:, w, :], in_=h0s[w * P:(w + 1) * P, :])

        # zero-init rotating gather buffers (stale reads on empty windows)
        for _ in range(6):
            g0 = gpo.tile([P, GCH, P], BF, tag="gq", name="gqz")
            nc.vector.memset(g0[:], 0.0)
        for _ in range(5):
            g1 = gpf.tile([P, FCH, DF], BF, tag="fgq", name="fgqz")
            nc.vector.memset(g1[:], 0.0)

        # pair-stride gather views (int16 indices address 2-row pairs)
        pv = xl_full.opt().rearrange("(a b) c -> a (b c)", b=2)
        v_ev, v_od = pv[:, 0:P], pv[:, P:2 * P]
        pvf = xlf_full.opt().rearrange("(a b) c -> a (b c)", b=2)
        vf_ev, vf_od = pvf[:, 0:DF], pvf[:, DF:2 * DF]

        # chunk-major table: AG chunk k outputs land contiguously
        agb = meta["agb"]
        agrow = [0]
        for (a, b) in agb:
            agrow.append(agrow[-1] + NCORES * (b - a) * P)

        # ---- all weights, loaded once ----
        wlL, wrL, blrL, brrL, attrL, lnwL, lnbL, biaL = \
            [], [], [], [], [], [], [], []
        for li in range(3):
            for k, (lst, dt, dd) in enumerate(
                    ((wlL, BF, wl_d), (wrL, BF, wr_d),
                     (blrL, F32, blr_d), (brrL, F32, brr_d),
                     (attrL, BF, attr_d), (lnwL, F32, lnw_d),
                     (lnbL, F32, lnb_d), (biaL, F32, bia_d))):
                t = cst.tile([P, P], dt, name=f"wt{li}x{k}")
                nc.sync.dma_start(out=t[:], in_=dd[li])
                lst.append(t)
        wlf = cst.tile([P, DF], BF, name="wlf_t")
        nc.sync.dma_start(out=wlf[:], in_=wlf_d[:, :])
        wrf = cst.tile([P, DF], BF, name="wrf_t")
        nc.sync.dma_start(out=wrf[:], in_=wrf_d[:, :])
        blfr = cst.tile([P, DF], F32, name="blf_t")
        nc.sync.dma_start(out=blfr[:], in_=blfr_d[:, :])
        brfr = cst.tile([P, DF], F32, name="brf_t")
        nc.sync.dma_start(out=brfr[:], in_=brfr_d[:, :])
        attfr = cst.tile([P, DF], BF, name="atf_t")
        nc.sync.dma_start(out=attfr[:], in_=attfr_d[:, :])
        biafr = cst.tile([P, P], F32, name="biaf_t")
        nc.sync.dma_start(out=biafr[:], in_=biafr_d[:, :])



        def prep_window(w, nxt, ps):
            """Transpose h window w; project for the NEXT layer; bounce out."""
            tp = ps.tile([P, P], BF, space="PSUM", tag="pt", name="ptb")
            nc.tensor.transpose(out=tp[:], in_=h_a[:, w, :],
                                identity=identb[:])
            nc.vector.tensor_copy(out=hT[:, w, :], in_=tp[:])
            if nxt == 3:
                xp = ps.tile([P, DF], F32, space="PSUM", tag="fxp",
                             name="fxp")
                nc.tensor.matmul(out=xp[:], lhsT=hT[:, w, :], rhs=wlf[:],
                                 start=True, stop=True)
                xs = wsp.tile([P, DF], BF, tag="fxs", name="fxs")
                nc.vector.tensor_tensor(out=xs[:], in0=xp[:], in1=blfr[:],
                                        op=OP.add)
                nc.sync.dma_start(out=xlf_b[w * P:(w + 1) * P, :], in_=xs[:])
            else:
                xp = ps.tile([P, P], F32, space="PSUM", tag="px", name="px")
                nc.tensor.matmul(out=xp[:], lhsT=hT[:, w, :], rhs=wlL[nxt],
                                 start=True, stop=True)
                xs = wsp.tile([P, P], BF, tag="p0xs", name="xs")
                nc.vector.tensor_tensor(out=xs[:], in0=xp[:], in1=blrL[nxt],
                                        op=OP.add)
                nc.sync.dma_start(out=xl_b[w * P:(w + 1) * P, :], in_=xs[:])

        def ag_chunks_after(w, nxt):
            """Fire any AllGather chunk that completes at window w."""
            src_t, dst_t = (xlf_b, xlf_full) if nxt == 3 else (xl_b, xl_full)
            for k, (a, b) in enumerate(agb):
                if b - 1 == w:
                    nrow = NCORES * (b - a) * P
                    nc.gpsimd.collective_compute(
                        "AllGather", OP.bypass, replica_groups=groups,
                        ins=[src_t.opt()[a * P:b * P, :]],
                        outs=[dst_t.opt()[agrow[k]:agrow[k] + nrow, :]])

        def gather_window(w, gq_of_tile, views, width, fch, chl):
            """Issue gather chunks for window w; fills gq_of_tile map."""
            out = []
            for (par, tlo, nt) in chl[w]:
                gq = (gpo.tile([P, GCH, P], BF, tag="gq", name="gq")
                      if width == P else
                      gpf.tile([P, FCH, DF], BF, tag="fgq", name="fgq"))
                nc.gpsimd.dma_gather(
                    gq[:, :nt, :], views[par],
                    idx_s[:, 8 * tlo:8 * (tlo + nt)],
                    nt * P, nt * P, width,
                    elem_step=2 * width, queue_num=qctr[0] % 2)
                qctr[0] += 1
                for t in range(nt):
                    gq_of_tile[tlo + t] = (gq, t)
                out.append(((par, tlo, nt), gq))
            return out

        # ------------------------------------------------------------------
        def hidden_layer(li, add_resid):
            wr, brr, attr = wrL[li], brrL[li], attrL[li]
            lnw, lnb, bia = lnwL[li], lnbL[li], biaL[li]

            with tc.tile_pool(name=f"ps{li}", bufs=1, space="PSUM") as ps:
                # PSUM: ep(1x2) + nmr(1) + dnm(1) + stats(1) + pt(1) + px(1)
                # (+fxp(1) when the next layer is the final one)
                if li == 0:
                    # initial projection for layer 0
                    for w in range(NW):
                        prep_window(w, 0, ps)
                        ag_chunks_after(w, 0)
                # P2: edge pipeline per window
                stp = ps.tile([2, GMAX], F32, space="PSUM", tag="stats",
                              name="stp")
                for w in range(NW):
                    T = int(Tw[w])
                    t0 = int(toff[w])
                    xrp = ps.tile([P, P], F32, space="PSUM", tag="px",
                                  name="xrp")
                    nc.tensor.matmul(out=xrp[:], lhsT=hT[:, w, :], rhs=wr[:],
                                     start=True, stop=True)
                    xr = wsp.tile([P, P], BF, tag="xr", name="xr")
                    nc.vector.tensor_tensor(out=xr[:], in0=xrp[:], in1=brr[:],
                                            op=OP.add)
                    spw = spo.tile([P, Tmax * P], BF, tag="sp", name="spw")
                    nc.sync.dma_start(out=spw[:, :T * P],
                                      in_=sp_d[:, t0 * P:(t0 + T) * P])
                    obw = spo.tile([P, Tmax * P], BF, tag="ob", name="obw")
                    nc.scalar.dma_start(out=obw[:, :T * P],
                                        in_=ob_d[:, t0 * P:(t0 + T) * P])
                    gqm = {}
                    gtiles = gather_window(w, gqm, (v_ev, v_od), P, GCH,
                                           chunks)
                    quads = []          # (gq buffer, rel_start, Q, abs_tile)
                    for (par, tlo, nt), gq in gtiles:
                        for a in range(0, nt, 4):
                            quads.append((gq, a, min(4, nt - a), tlo + a))

                    nmr = ps.tile([P, P], F32, space="PSUM", tag="nmr",
                                  name="nmr")
                    dnm = ps.tile([P, HEADS], F32, space="PSUM", tag="dnm",
                                  name="dnm")

                    for qi, (gqb, a, Q, tabs) in enumerate(quads):
                        ts = tabs - t0
                        ep = ps.tile([P, 4 * P], F32, space="PSUM", tag="ep",
                                     bufs=2, name="ep")
                        for t in range(Q):
                            nc.tensor.matmul(
                                out=ep[:, t * P:(t + 1) * P],
                                lhsT=spw[:, (ts + t) * P:(ts + t + 1) * P],
                                rhs=xr[:], start=True, stop=True)
                        gqv = gqb[:, a:a + Q, :]
                        tq = wsp.tile([P, 4 * P], BF, tag="tq", name="tq")
                        nc.vector.tensor_tensor(
                            out=tq[:, :Q * P], in0=ep[:, :Q * P],
                            in1=gqv.rearrange("p t c -> p (t c)"), op=OP.add)
                        rl = wsp.tile([P, 4 * P], BF, tag="rl", name="rl")
                        nc.scalar.activation(out=rl[:, :Q * P],
                                             in_=tq[:, :Q * P], func=AF.Relu,
                                             scale=-(1.0 - NEG))
                        ea = wsp.tile([P, 4 * P], BF, tag="ea", name="ea")
                        nc.vector.tensor_tensor(out=ea[:, :Q * P],
                                                in0=tq[:, :Q * P],
                                                in1=rl[:, :Q * P], op=OP.add)
                        lg = wsp.tile([P, 4 * P], BF, tag="lg", name="lg")
                        nc.vector.tensor_tensor(
                            out=lg[:, :Q * P], in0=ea[:, :Q * P],
                            in1=attr[:, None, :].to_broadcast([P, Q, P]),
                            op=OP.mult)
                        lgr = wsp.tile([P, 4 * HEADS], BF, tag="lgr",
                                       name="lgr")
                        nc.vector.tensor_reduce(
                            out=lgr[:, :Q * HEADS],
                            in_=lg[:].rearrange("p (t h c) -> p (t h) c",
                                                h=HEADS, c=CH)[:, :Q * HEADS, :],
                            axis=AX.X, op=OP.add)
                        wq = wsp.tile([P, 4 * HEADS], BF, tag="wq", name="wq")
                        nc.scalar.activation(out=wq[:, :Q * HEADS],
                                             in_=lgr[:, :Q * HEADS], func=AF.Exp)
                        mm = wsp.tile([P, 4, HEADS, CH], BF, tag="mm",
                                      name="mmt")
                        nc.vector.tensor_tensor(
                            out=mm[:, :Q, :, :],
                            in0=gqv.rearrange("p t (h c) -> p t h c",
                                              h=HEADS, c=CH),
                            in1=wq[:].rearrange("p (t h) -> p t h", h=HEADS)
                                [:, :Q, :, None].to_broadcast([P, Q, HEADS, CH]),
                            op=OP.mult)
                        for t in range(Q):
                            first = (qi == 0 and t == 0)
                            last = (qi == len(quads) - 1 and t == Q - 1)
                            ob_t = obw[:, (ts + t) * P:(ts + t + 1) * P]
                            nc.tensor.matmul(
                                out=nmr[:], lhsT=ob_t, rhs=mm[:, t, :, :],
                                start=first, stop=last)
                            nc.tensor.matmul(
                                out=dnm[:], lhsT=ob_t,
                                rhs=wq[:, t * HEADS:(t + 1) * HEADS],
                                start=first, stop=last)

                    # window flush (node-major)
                    if not quads:
                        nc.vector.tensor_copy(out=htmp[:, w, :], in_=bia[:])
                    if quads:
                        rd = wsp.tile([P, HEADS], F32, tag="rd", name="rd")
                        nc.vector.tensor_scalar(out=rd[:], in0=dnm[:],
                                                scalar1=1e-16, scalar2=None,
                                                op0=OP.add)
                        nc.vector.reciprocal(out=rd[:], in_=rd[:])
                        oT = wsp.tile([P, HEADS, CH], F32, tag="oT", name="oT")
                        nc.vector.tensor_tensor(
                            out=oT[:],
                            in0=nmr[:].rearrange("p (h c) -> p h c",
                                                 h=HEADS, c=CH),
                            in1=rd[:, :, None].to_broadcast([P, HEADS, CH]),
                            op=OP.mult)
                        nc.vector.tensor_tensor(
                            out=htmp[:, w, :],
                            in0=oT[:].rearrange("p h c -> p (h c)"),
                            in1=bia[:], op=OP.add)
                    # stats: [row-sum | row-sumsq] -> per-graph (PSUM accum)
                    s12 = wsp.tile([P, 2], F32, tag="s12", name="s12")
                    nc.vector.tensor_reduce(out=s12[:, 0:1], in_=htmp[:, w, :],
                                            axis=AX.X, op=OP.add)
                    sqj = wsp.tile([P, P], F32, tag="sqj", name="sqj")
                    nc.scalar.activation(out=sqj[:], in_=htmp[:, w, :],
                                         func=AF.Square, accum_out=s12[:, 1:2])
                    gm = wsp.tile([P, GMAX], F32, tag="gm", name="gm")
                    nc.sync.dma_start(out=gm[:], in_=gmat_d[w])
                    nc.tensor.matmul(out=stp[:, :], lhsT=s12[:],
                                     rhs=gm[:], start=(w == 0),
                                     stop=(w == NW - 1))

                # P3: stats -> mean/rstd -> normalize + elu
                sts = wsp.tile([2, GMAX], F32, tag="sts", name="sts")
                nc.vector.tensor_copy(out=sts[:], in_=stp[:])
                nc.sync.dma_start(out=st_b[:, :], in_=sts[:])
                nc.gpsimd.collective_compute(
                    "AllReduce", OP.add, replica_groups=groups,
                    ins=[st_b.opt()], outs=[st_o.opt()])
                stg1 = wsp.tile([1, GMAX], F32, tag="stg1", name="stg1")
                nc.sync.dma_start(out=stg1[:], in_=st_o[0:1, :])
                stg2 = wsp.tile([1, GMAX], F32, tag="stg2", name="stg2")
                nc.sync.dma_start(out=stg2[:], in_=st_o[1:2, :])
                mean = wsp.tile([1, GMAX], F32, tag="mean", name="mean")
                nc.vector.tensor_tensor(out=mean[:], in0=stg1[:],
                                        in1=invd[:], op=OP.mult)
                ex2 = wsp.tile([1, GMAX], F32, tag="ex2", name="ex2")
                nc.vector.tensor_tensor(out=ex2[:], in0=stg2[:],
                                        in1=invd[:], op=OP.mult)
                msq = wsp.tile([1, GMAX], F32, tag="msq", name="msq")
                nc.scalar.activation(out=msq[:], in_=mean[:], func=AF.Square)
                var = wsp.tile([1, GMAX], F32, tag="var", name="var")
                nc.vector.tensor_tensor(out=var[:], in0=ex2[:], in1=msq[:],
                                        op=OP.subtract)
                sd = wsp.tile([1, GMAX], F32, tag="sd", name="sd")
                nc.scalar.activation(out=sd[:], in_=var[:], func=AF.Sqrt,
                                     bias=epsc[0:1, 0:1])
                rstd = wsp.tile([1, GMAX], F32, tag="rstd", name="rstd")
                nc.vector.reciprocal(out=rstd[:], in_=sd[:])
                nmr2 = wsp.tile([1, GMAX], F32, tag="nmr2", name="nm2")
                nc.vector.tensor_tensor(out=nmr2[:], in0=mean[:], in1=rstd[:],
                                        op=OP.mult)
                nc.vector.tensor_scalar(out=nmr2[:], in0=nmr2[:], scalar1=-1.0,
                                        scalar2=None, op0=OP.mult)
                t1 = ps.tile([P, P], F32, space="PSUM", tag="pt", name="t1")
                nc.tensor.transpose(out=t1[0:GMAX, 0:1], in_=nmr2[:],
                                    identity=ident[0:1, 0:1])
                t2 = ps.tile([P, P], F32, space="PSUM", tag="px", name="t2")
                nc.tensor.transpose(out=t2[0:GMAX, 0:1], in_=rstd[:],
                                    identity=ident[0:1, 0:1])
                nrcol = wsp.tile([GMAX, 2], F32, tag="nrcol", name="nrc")
                nc.vector.tensor_copy(out=nrcol[:, 0:1], in_=t1[0:GMAX, 0:1])
                nc.vector.tensor_copy(out=nrcol[:, 1:2], in_=t2[0:GMAX, 0:1])

                for w in range(NW):
                    gmT = wsp.tile([GMAX, P], F32, tag="gmT", name="gmT")
                    nc.sync.dma_start(out=gmT[:], in_=gmatT_d[w])
                    mw = ps.tile([P, P], F32, space="PSUM", tag="pt",
                                 name="mw")
                    nc.tensor.matmul(out=mw[:, 0:2], lhsT=gmT[:], rhs=nrcol[:],
                                     start=True, stop=True)
                    mws = wsp.tile([P, 2], F32, tag="mws", name="mws")
                    nc.vector.tensor_copy(out=mws[:], in_=mw[:, 0:2])
                    xn = wsp.tile([P, P], F32, tag="xn", name="xn")
                    nc.scalar.activation(out=xn[:], in_=htmp[:, w, :],
                                         func=AF.Identity, scale=mws[:, 1:2],
                                         bias=mws[:, 0:1])
                    nc.vector.tensor_tensor(out=xn[:], in0=xn[:], in1=lnw[:],
                                            op=OP.mult)
                    nc.vector.tensor_tensor(out=xn[:], in0=xn[:], in1=lnb[:],
                                            op=OP.add)
                    # elu = max(x,0) + exp(min(x,0)) - 1
                    mn = wsp.tile([P, P], F32, tag="mn", name="mn")
                    nc.vector.tensor_scalar(out=mn[:], in0=xn[:], scalar1=0.0,
                                            scalar2=None, op0=OP.min)
                    nc.scalar.activation(out=mn[:], in_=mn[:], func=AF.Exp)
                    mx = wsp.tile([P, P], F32, tag="mx", name="mx")
                    nc.vector.tensor_scalar(out=mx[:], in0=xn[:], scalar1=0.0,
                                            scalar2=None, op0=OP.max)
                    nc.vector.tensor_tensor(out=mx[:], in0=mx[:], in1=mn[:],
                                            op=OP.add)
                    if add_resid:
                        nc.vector.tensor_scalar(out=mx[:], in0=mx[:],
                                                scalar1=1.0, scalar2=None,
                                                op0=OP.subtract)
                        rt = wsp.tile([P, P], F32, tag="rt", name="rt")
                        nc.sync.dma_start(out=rt[:],
                                          in_=rs[w * P:(w + 1) * P, :])
                        nc.vector.tensor_tensor(out=h_a[0:W, w, :],
                                                in0=mx[0:W, :],
                                                in1=rt[0:W, :], op=OP.add)
                    else:
                        nc.vector.tensor_scalar(out=h_a[0:W, w, :],
                                                in0=mx[0:W, :],
                                                scalar1=1.0, scalar2=None,
                                                op0=OP.subtract)
                    prep_window(w, li + 1, ps)
                    ag_chunks_after(w, li + 1)

        # ------------------------------------------------------------------
        def final_layer():
            with tc.tile_pool(name="psf", bufs=1, space="PSUM") as ps:
                # PSUM: fep(1x2) + fnm(1) + fdnm(1)
                for w in range(NW):
                    T = int(Tw[w])
                    t0 = int(toff[w])
                    xrp = ps.tile([P, DF], F32, space="PSUM", tag="fep",
                                  bufs=2, name="fxrp")
                    nc.tensor.matmul(out=xrp[:], lhsT=hT[:, w, :], rhs=wrf[:],
                                     start=True, stop=True)
                    xr = wsp.tile([P, DF], BF, tag="fxr", bufs=1, name="fxr")
                    nc.vector.tensor_tensor(out=xr[:], in0=xrp[:], in1=brfr[:],
                                            op=OP.add)
                    spw = spo.tile([P, Tmax * P], BF, tag="sp", name="fspw")
                    nc.sync.dma_start(out=spw[:, :T * P],
                                      in_=sp_d[:, t0 * P:(t0 + T) * P])
                    obw = spo.tile([P, Tmax * P], BF, tag="ob", name="fobw")
                    nc.scalar.dma_start(out=obw[:, :T * P],
                                        in_=ob_d[:, t0 * P:(t0 + T) * P])
                    gqm = {}
                    gather_window(w, gqm, (vf_ev, vf_od), DF, FCH, fchunks)

                    fnm = ps.tile([P, DF], F32, space="PSUM", tag="fnm",
                                  name="fnm")
                    dnm = ps.tile([P, HEADS], F32, space="PSUM", tag="fdnm",
                                  name="fdnm")

                    for t in range(T):
                        gqb, rt_ = gqm[t0 + t]
                        gqv = gqb[:, rt_, :]
                        ep = ps.tile([P, DF], F32, space="PSUM", tag="fept",
                                     bufs=2, name="fept")
                        nc.tensor.matmul(out=ep[:],
                                         lhsT=spw[:, t * P:(t + 1) * P],
                                         rhs=xr[:], start=True, stop=True)
                        cp = wsp.tile([P, DF], BF, tag="fcp", name="fcp")
                        nc.scalar.activation(out=cp[:], in_=ep[:],
                                             func=AF.Identity)
                        tq = wsp.tile([P, DF], BF, tag="ftq", name="ftq")
                        nc.vector.tensor_tensor(out=tq[:], in0=cp[:],
                                                in1=gqv, op=OP.add)
                        rl = wsp.tile([P, DF], BF, tag="frl", name="frl")
                        nc.scalar.activation(out=rl[:], in_=tq[:],
                                             func=AF.Relu,
                                             scale=-(1.0 - NEG))
                        ea = wsp.tile([P, DF], BF, tag="fea", name="fea")
                        nc.vector.tensor_tensor(out=ea[:], in0=tq[:],
                                                in1=rl[:], op=OP.add)
                        lg = wsp.tile([P, DF], BF, tag="flg", name="flg")
                        nc.vector.tensor_tensor(out=lg[:], in0=ea[:],
                                                in1=attfr[:], op=OP.mult)
                        lgr = wsp.tile([P, HEADS], BF, tag="flgr",
                                       name="flgr")
                        nc.vector.tensor_reduce(
                            out=lgr[:],
                            in_=lg[:].rearrange("p (h c) -> p h c", h=HEADS,
                                                c=P),
                            axis=AX.X, op=OP.add)
                        wq = wsp.tile([P, HEADS], BF, tag="fwq", name="fwq")
                        nc.scalar.activation(out=wq[:], in_=lgr[:], func=AF.Exp)
                        mm = wsp.tile([P, HEADS, P], BF, tag="fmm", bufs=2,
                                      name="fmm")
                        nc.vector.tensor_tensor(
                            out=mm[:],
                            in0=gqv.rearrange("p (h c) -> p h c", h=HEADS,
                                              c=P),
                            in1=wq[:, :, None].to_broadcast([P, HEADS, P]),
                            op=OP.mult)
                        ob_t = obw[:, t * P:(t + 1) * P]
                        nc.tensor.matmul(
                            out=fnm[:], lhsT=ob_t,
                            rhs=mm[:].rearrange("p h c -> p (h c)"),
                            start=(t == 0), stop=(t == T - 1))
                        nc.tensor.matmul(out=dnm[:], lhsT=ob_t, rhs=wq[:],
                                         start=(t == 0), stop=(t == T - 1))

                    # flush: out = bias + sum_h numer[n,h,:]*(0.25/denom[n,h])
                    rd = wsp.tile([P, HEADS], F32, tag="rd", name="rdf")
                    nc.vector.tensor_scalar(out=rd[:], in0=dnm[:],
                                            scalar1=1e-16, scalar2=None,
                                            op0=OP.add)
                    nc.vector.reciprocal(out=rd[:], in_=rd[:])
                    nc.vector.tensor_scalar(out=rd[:], in0=rd[:],
                                            scalar1=1.0 / HEADS, scalar2=None,
                                            op0=OP.mult)
                    fns = wsp.tile([P, DF], BF, tag="fns", name="fns")
                    nc.scalar.activation(out=fns[:], in_=fnm[:],
                                         func=AF.Identity)
                    sc = wsp.tile([P, HEADS, P], F32, tag="sc", bufs=1,
                                  name="sc")
                    nc.vector.tensor_tensor(
                        out=sc[:],
                        in0=fns[:].rearrange("p (h c) -> p h c", h=HEADS, c=P),
                        in1=rd[:, :, None].to_broadcast([P, HEADS, P]),
                        op=OP.mult)
                    acc = wsp.tile([P, P], F32, tag="acc", name="acc")
                    nc.vector.tensor_reduce(
                        out=acc[:], in_=sc[:].rearrange("p h c -> p c h"),
                        axis=AX.X, op=OP.add)
                    nc.vector.tensor_tensor(out=acc[:], in0=acc[:],
                                            in1=biafr[:], op=OP.add)
                    nc.sync.dma_start(out=out_d[w * P:(w + 1) * P, :],
                                      in_=acc[:])

        # ---- the 4 layers ----
        with nc.allow_low_precision(reason="bf16 edge intermediates; "
                                    "softmax tolerates it (rel-err gate)"):
            hidden_layer(0, add_resid=False)
            hidden_layer(1, add_resid=True)
            hidden_layer(2, add_resid=False)
            final_layer()

    nc.compile()
    return nc


# ----------------------------------------------------------------------------
# Host-side driver
# ----------------------------------------------------------------------------

def _rep(v, rows=P):
    v = np.asarray(v, np.float32).reshape(-1)
    return np.broadcast_to(v, (rows, v.shape[0])).copy()


def make_in_maps(meta, inputs):
    NPAD, TT, NW = meta["NPAD"], meta["TT"], meta["NW"]
    lo, hi = meta["lo"], meta["hi"]
    x = np.asarray(inputs["x"], np.float32)
    resid = np.asarray(inputs["residual"], np.float32)
    ew = np.asarray(inputs["edge_weight"], np.float32)

    att = np.asarray(inputs["att"], np.float32)        # (3, H, C)
    attf = np.asarray(inputs["att_f"], np.float32)     # (H, DOUT)
    We = np.asarray(inputs["We"], np.float32)          # (3, 1, DHID)
    Wef = np.asarray(inputs["We_f"], np.float32)       # (1, H*DOUT)

    brr = np.stack([_rep(inputs["br"][i]) for i in range(3)])
    for i in range(3):
        brr[i, P - 1, :] = We[i, 0, :]
    brfr = _rep(inputs["br_f"])
    brfr[P - 1, :] = Wef[0, :]

    common = dict(
        invd=meta["invd"].astype(np.float32),
        idf=np.eye(P, dtype=np.float32),
        idb=np.eye(P, dtype=np.float32).astype(ml_dtypes.bfloat16),
        wl=np.asarray(inputs["Wl"], np.float32).astype(ml_dtypes.bfloat16),
        wr=np.asarray(inputs["Wr"], np.float32).astype(ml_dtypes.bfloat16),
        blr=np.stack([_rep(inputs["bl"][i]) for i in range(3)]),
        brr=brr,
        attr=np.stack([_rep(att[i]) for i in range(3)]).astype(ml_dtypes.bfloat16),
        lnw=np.stack([_rep(inputs["ln_w"][i]) for i in range(3)]),
        lnb=np.stack([_rep(inputs["ln_b"][i]) for i in range(3)]),
        bia=np.stack([_rep(inputs["bias"][i]) for i in range(3)]),
        wlf=np.asarray(inputs["Wl_f"], np.float32).astype(ml_dtypes.bfloat16),
        wrf=np.asarray(inputs["Wr_f"], np.float32).astype(ml_dtypes.bfloat16),
        blfr=_rep(inputs["bl_f"]),
        brfr=brfr,
        attfr=_rep(attf).astype(ml_dtypes.bfloat16),
        biafr=_rep(inputs["bias_f"]),
    )

    in_maps = []
    for c in range(NCORES):
        n = int(hi[c] - lo[c])
        r = np.arange(n)
        prow = (r // W) * P + (r % W)
        h0s = np.zeros((NPAD, P), ml_dtypes.bfloat16)
        h0s[prow] = x[lo[c]:hi[c]].astype(ml_dtypes.bfloat16)
        rss = np.zeros((NPAD, P), np.float32)
        rss[prow] = resid[lo[c]:hi[c]]

        pc = meta["percore"][c]
        col = pc["tile"] * P
        sp = np.zeros((P, TT * P), ml_dtypes.bfloat16)
        sp[pc["dstl"], col + pc["pos"]] = 1.0
        sp[P - 1, col + pc["pos"]] = ew[pc["eid"]].astype(ml_dtypes.bfloat16)
        ob = np.zeros((P, TT * P), ml_dtypes.bfloat16)
        ob[pc["pos"], col + pc["dstl"]] = 1.0
        idx16 = np.zeros((16, 8 * TT), np.int16)
        icol = 8 * pc["base"] + pc["j"] // 16
        irow = pc["j"] % 16
        idx16[irow, icol] = pc["ival"]
        idx = np.tile(idx16, (8, 1))

        in_maps.append(dict(
            h0s=h0s, rs=rss, idx=idx, sp=sp, ob=ob,
            gmat=meta["gmat"][c], gmatT=meta["gmatT"][c],
            **common))
    return in_maps


def assemble(meta, results):
    N = meta["N"]
    lo, hi = meta["lo"], meta["hi"]
    out = np.zeros((N, P), np.float32)
    for c in range(NCORES):
        n = int(hi[c] - lo[c])
        r = np.arange(n)
        out[lo[c]:hi[c]] = results[c]["out"][(r // W) * P + (r % W)]
    return out


_CACHE = {}


def kernel(**inputs):
    ei = np.asarray(inputs["edge_index"])
    bt = np.asarray(inputs["batch"])
    key = (ei.shape, bt.shape, hash(ei.tobytes()), hash(bt.tobytes()))
    if key not in _CACHE:
        meta = build_meta(ei, bt)
        nc = build_program(meta)
        _CACHE[key] = (meta, nc)
    meta, nc = _CACHE[key]
    in_maps = make_in_maps(meta, inputs)
    res = run_bass_kernel_spmd(nc, in_maps, list(range(NCORES)))
    return assemble(meta, res.results)
